# revision 7
# baseline (speedup 1.0000x reference)
"""Trainium2 Bass kernel for BasicQuantConv2d (sync-BN + HWGQ + gauss-quant + 3x3 conv).

Strategy (8 NeuronCores, data-parallel over batch; 4 images/core):
  - BN batch stats: per-core bn_stats/bn_aggr -> (mean, E[x^2]) payload,
    AllGather across the 8 cores + local 8-way sum (sync-BN), exact /8.
  - BN + HWGQ folds to ia = RNE_round(clip(x*s_c + b_c, 0, 3)) in {0..3};
    RNE via the fp32 magic constant 1.5*2^23 (matches jnp.round).
  - gauss_quantize(w) == iw * (step/2) with iw in {-3,-1,1,3}; std(w)
    computed on-device; conv in fp8 (exact integer arithmetic), 5 passes
    per 8-row chunk (3 DoubleRow vertical tap pairs, 1 DoubleRow
    horizontal pair, 1 single), 7 PSUM banks per image.

Engine assignment (v2; driven by microbenchmarked per-engine rates and the
DVE/GpSimd shared-port lock):
  - Act:  per-image affine u = s*x + b (Identity activation, per-channel
          scale/bias APs) interleaved with the PSUM drains (*alpha -> fp16).
  - DVE:  bn_stats (REGULAR mode; overlaps Pool), clip (min 3, max 0)
          in-place on u (2X_2PORT), payload fixup; rsqrt chain is
          integer-magic + Newton (no Act round-trip).
  - Pool: gather-in reduce + scale/bias chain + the RNE rounds
          (+M, -M -> fp8 padded a_t), kept off DVE's 2-port windows.
  - Collective payload/gather DMAs ride the DVE/Pool queues (issue-after-
    produce), the AllGather itself the gpsimd queue.

3-stage software pipeline: window w runs conv(w-2) on a_t parity (w-2)%2,
prep(w-1) writing parity (w-1)%2, and front(w) loads+stats+collective.
a_t is double-buffered so rounds never WAR-wait on convs; x tiles are
whole-image [128, 3136] loads, double-buffered (lifetime 2 windows).
Output DMAs move fp16 (host casts back; <=2^-11 added relative error).
"""

import numpy as np

import concourse.bacc as bacc
import concourse.bass as bass
import concourse.tile as tile
from concourse import mybir
from concourse.masks import make_identity

N_CORES = 8
IMG = 4            # images per core
C = 128            # channels (= partitions)
HW = 56
S = HW * HW        # 3136 pixels per image
G = 448            # stats granule (8 image rows)
NT = HW // 8       # 7 granules / PSUM chunks per image
PR = 58            # padded rows
PCW = 64           # padded row width (interior cols 2..57; 64B pair-step)
R = 8              # output rows per matmul tile
NFREE = R * HW     # 448 matmul free dim

HWGQ_STEP = 0.538
GAUSS = 0.996
BN_EPS = 1e-3
MAGIC = float(np.float32(1.5 * 2**23))
RSQRT_MAGIC = 0x5F3759DF
NW = 128 * 128 * 9

fp32 = mybir.dt.float32
fp16 = mybir.dt.float16
fp8 = mybir.dt.float8e4
i32 = mybir.dt.int32

_CACHE = {}


def _emit_prelude(nc, tc, pools, params):
    """Iteration-invariant: weight quantization, pad memsets, gamma/beta."""
    xp, apadp, wp, tmpp, outp, smallp, psump, psmallp, dramp = pools
    x_d, gamma_d, beta_d, w_d, y_d = params
    AF = mybir.ActivationFunctionType
    OP = mybir.AluOpType

    w_sb = wp.tile([C, 128 * 9], fp32)
    nc.sync.dma_start(out=w_sb[:], in_=w_d.ap())

    ident = smallp.tile([C, 128], fp32, tag="ident")
    make_identity(nc, ident[:])

    psm = psmallp.tile([C, 512], fp32, tag="psm", name="psm")

    # transpose each tap: wT[ci, slot, co]; slots pair (0,kw)+(1,kw) and
    # (2,0)+(2,1) adjacently for DoubleRow; (2,2) last.
    SLOT = {(0, 0): 0, (1, 0): 1, (0, 1): 2, (1, 1): 3,
            (0, 2): 4, (1, 2): 5, (2, 0): 6, (2, 1): 7, (2, 2): 8}
    wT = wp.tile([C, 9, 128], fp32)
    w3 = w_sb[:].rearrange("p (ci t) -> p ci t", t=9)
    for t in range(9):
        kh, kw = divmod(t, 3)
        pt = psm[:, (t % 2) * 128:(t % 2) * 128 + 128]
        nc.tensor.transpose(pt, w3[:, :, t], ident[:])
        nc.scalar.copy(out=wT[:, SLOT[(kh, kw)], :], in_=pt)

    # global sum / sumsq of w: ScalarE accum_out row-sums + ones-matmul bcast
    uw = wp.tile([C, 9, 128], fp32)
    w2_sb = uw[:].rearrange("p a b -> p (a b)")
    rsums = smallp.tile([C, 2], fp32, tag="rsums")
    nc.scalar.activation(out=w2_sb, in_=w_sb[:], func=AF.Identity,
                         accum_out=rsums[:, 0:1])
    nc.scalar.activation(out=w2_sb, in_=w_sb[:], func=AF.Square,
                         accum_out=rsums[:, 1:2])
    ones = smallp.tile([C, 128], fp32, tag="ones")
    nc.vector.memset(ones[:], 1.0)
    pg = psm[:, 256:384]
    nc.tensor.matmul(pg[:, 0:2], lhsT=ones[:], rhs=rsums[:], start=True, stop=True)
    gs = smallp.tile([C, 2], fp32, tag="gs")
    nc.vector.tensor_copy(gs[:], pg[:, 0:2])

    # wvar = E[w^2] - E[w]^2 ; rw = rsqrt(wvar) Newton-refined
    wmean = smallp.tile([C, 1], fp32, tag="wmean")
    wvar = smallp.tile([C, 1], fp32, tag="wvar")
    nc.vector.tensor_scalar_mul(wmean[:], gs[:, 0:1], 1.0 / NW)
    nc.vector.tensor_scalar_mul(wvar[:], gs[:, 1:2], 1.0 / NW)
    wm2 = smallp.tile([C, 1], fp32, tag="wm2")
    nc.vector.tensor_mul(wm2[:], wmean[:], wmean[:])
    nc.vector.tensor_sub(wvar[:], wvar[:], wm2[:])

    rw = smallp.tile([C, 1], fp32, tag="rw")
    nc.scalar.activation(out=rw[:], in_=wvar[:], func=AF.Sqrt)
    nc.vector.reciprocal(out=rw[:], in_=rw[:])
    tN = smallp.tile([C, 1], fp32, tag="tN")
    for _ in range(2):
        nc.vector.tensor_mul(tN[:], rw[:], rw[:])
        nc.vector.tensor_mul(tN[:], wvar[:], tN[:])
        nc.vector.tensor_scalar(tN[:], tN[:], -0.5, 1.5, OP.mult, OP.add)
        nc.vector.tensor_mul(rw[:], rw[:], tN[:])

    inv_step = smallp.tile([C, 1], fp32, tag="inv_step")
    nc.vector.tensor_scalar_mul(inv_step[:], rw[:], 1.0 / GAUSS)
    # alpha = 0.538 * step/2 = (0.538*0.996/2) * wvar * rw
    alpha = smallp.tile([C, 1], fp32, tag="alpha")
    nc.vector.tensor_mul(alpha[:], wvar[:], rw[:])
    nc.vector.tensor_scalar_mul(alpha[:], alpha[:], HWGQ_STEP * GAUSS / 2.0)

    # quantize transposed weights -> iw in {-3,-1,1,3} (fp8)
    nc.gpsimd.tensor_scalar(uw[:], wT[:], inv_step[:], 0.5, OP.mult, OP.add)
    nc.gpsimd.tensor_scalar(uw[:], uw[:], MAGIC, MAGIC, OP.add, OP.subtract)
    nc.gpsimd.tensor_scalar(uw[:], uw[:], 2.0, -1.0, OP.mult, OP.add)
    wq = wp.tile([C, 9, 128], fp8)
    nc.gpsimd.tensor_scalar(wq[:], uw[:], 3.0, -3.0, OP.min, OP.max)

    # gamma/beta
    gb = smallp.tile([C, 2], fp32, tag="gb")
    gamma_ap = gamma_d.ap().rearrange("(p one) -> p one", one=1)
    beta_ap = beta_d.ap().rearrange("(p one) -> p one", one=1)
    nc.sync.dma_start(out=gb[:, 0:1], in_=gamma_ap)
    nc.sync.dma_start(out=gb[:, 1:2], in_=beta_ap)

    # int constant for the rsqrt magic seed
    magic_i = smallp.tile([C, 1], i32, tag="magic_i")
    nc.vector._memset_packed(magic_i[:], RSQRT_MAGIC)

    # padded fp8 activation tiles, double-buffered by iteration parity:
    # interiors rewritten every iteration, borders stay zero forever
    a_t = [[apadp.tile([C, PR, PCW], fp8, tag=f"a{par}_{i}",
                       name=f"a_t{par}_{i}") for i in range(IMG)]
           for par in range(2)]
    for par in range(2):
        for i in range(IMG):
            nc.gpsimd.memset(a_t[par][i][:], 0.0)

    return dict(ones=ones, wq=wq, alpha=alpha, gb=gb, a_t=a_t, psm=psm,
                magic_i=magic_i)


def _emit_front_loads(nc, pools, params, it):
    """x loads for iteration `it`: one whole-image DMA per image (sync)."""
    xp = pools[0]
    x_d = params[0]
    xH = [xp.tile([C, S], fp32, tag=f"x{i}", name=f"x{it}_{i}")
          for i in range(IMG)]
    for i in range(IMG):
        nc.sync.dma_start(out=xH[i][:], in_=x_d.ap()[i])
    return dict(xH=xH)


def _emit_front_stats(nc, pools, params, st, it):
    """bn_stats + payload + AllGather dispatch for iteration `it`."""
    xp, apadp, wp, tmpp, outp, smallp, psump, psmallp, dramp = pools
    OP = mybir.AluOpType
    xH = st["xH"]

    stats = smallp.tile([C, IMG * NT, 6], fp32, tag="stats", name="stats")
    for i in range(IMG):
        for g in range(NT):
            nc.vector.bn_stats(out=stats[:, i * NT + g, :],
                               in_=xH[i][:, g * G:(g + 1) * G])
    # payload: (mean, E[x^2]); E[x^2] = var + mean^2 (in-place)
    pay = smallp.tile([C, 2], fp32, tag="pay", name="pay")
    nc.vector.bn_aggr(out=pay[:], in_=stats[:])
    m2 = smallp.tile([C, 1], fp32, tag="m2", name="m2")
    nc.vector.tensor_mul(m2[:], pay[:, 0:1], pay[:, 0:1])
    nc.vector.tensor_add(pay[:, 1:2], pay[:, 1:2], m2[:])

    cc_in = dramp.tile([C, 2], fp32)
    cc_gath = dramp.tile([N_CORES, C, 2], fp32)
    # payload out on the Pool queue (idle by stats-end; rounds are long done)
    nc.gpsimd.dma_start(out=cc_in[:], in_=pay[:])
    nc.gpsimd.collective_compute(
        "AllGather",
        OP.bypass,
        replica_groups=[list(range(N_CORES))],
        ins=[cc_in.opt()],
        outs=[cc_gath.opt()],
    )
    st["cc_gath"] = cc_gath


def _emit_prep_chain(nc, pools, pre, st, it):
    """gather-in + global scale/bias chain for iteration `it` (Pool only,
    so the Act queue never head-blocks on the collective)."""
    xp, apadp, wp, tmpp, outp, smallp, psump, psmallp, dramp = pools
    OP = mybir.AluOpType
    gb, magic_i = pre["gb"], pre["magic_i"]

    g_all = smallp.tile([C, N_CORES, 2], fp32, tag="g_all", name="g_all")
    nc.gpsimd.dma_start(out=g_all[:],
                        in_=st["cc_gath"][:].rearrange("r p t -> p r t"))
    # local 8-way sum: pairwise tree on Pool (same order on all cores)
    g4 = smallp.tile([C, 4, 2], fp32, tag="g4", name="g4")
    nc.gpsimd.tensor_tensor(out=g4[:], in0=g_all[:, 0:4, :],
                            in1=g_all[:, 4:8, :], op=OP.add)
    g2 = smallp.tile([C, 2, 2], fp32, tag="g2", name="g2")
    nc.gpsimd.tensor_tensor(out=g2[:], in0=g4[:, 0:2, :],
                            in1=g4[:, 2:4, :], op=OP.add)
    g_sum = smallp.tile([C, 2], fp32, tag="g_sum", name="g_sum")
    nc.gpsimd.tensor_tensor(out=g_sum[:], in0=g2[:, 0, :],
                            in1=g2[:, 1, :], op=OP.add)

    me = smallp.tile([C, 2], fp32, tag="me", name="me")
    nc.gpsimd.tensor_scalar_mul(me[:], g_sum[:], 0.125)
    meanv = me[:, 0:1]
    vge = smallp.tile([C, 1], fp32, tag="vge", name="vge")
    gm2 = smallp.tile([C, 1], fp32, tag="gm2", name="gm2")
    nc.gpsimd.tensor_mul(gm2[:], meanv, meanv)
    nc.gpsimd.tensor_scalar(vge[:], me[:, 1:2], gm2[:], BN_EPS,
                            OP.subtract, OP.add)

    # rsqrt(vge) via int32 magic seed (DVE: Pool lacks shift ops) + Newton
    rx = smallp.tile([C, 1], fp32, tag="rx", name="rx")
    rx_i = rx[:].bitcast(i32)
    nc.vector.tensor_scalar(rx_i, vge[:].bitcast(i32), 1, None,
                            OP.logical_shift_right)
    nc.vector.tensor_tensor(out=rx_i, in0=magic_i[:], in1=rx_i,
                            op=OP.subtract)
    tX = smallp.tile([C, 1], fp32, tag="tX", name="tX")
    for _ in range(3):
        nc.gpsimd.tensor_mul(tX[:], rx[:], rx[:])
        nc.gpsimd.tensor_mul(tX[:], vge[:], tX[:])
        nc.gpsimd.tensor_scalar(tX[:], tX[:], -0.5, 1.5, OP.mult, OP.add)
        nc.gpsimd.tensor_mul(rx[:], rx[:], tX[:])

    # s = gamma * rsqrt / 0.538 ; b = (beta - mean*gamma*rsqrt) / 0.538
    s_q = smallp.tile([C, 1], fp32, tag="s_q", name="s_q")
    b_q = smallp.tile([C, 1], fp32, tag="b_q", name="b_q")
    ta = smallp.tile([C, 1], fp32, tag="ta", name="ta")
    nc.gpsimd.tensor_mul(ta[:], gb[:, 0:1], rx[:])
    nc.gpsimd.tensor_scalar_mul(s_q[:], ta[:], 1.0 / HWGQ_STEP)
    tb = smallp.tile([C, 1], fp32, tag="tb", name="tb")
    nc.gpsimd.tensor_mul(tb[:], meanv, ta[:])
    nc.gpsimd.tensor_scalar(b_q[:], gb[:, 1:2], tb[:], 1.0 / HWGQ_STEP,
                            OP.subtract, OP.mult)
    st["s_q"], st["b_q"] = s_q, b_q


def _emit_prep_image(nc, pools, pre, st, it, i):
    """Affine (Act) + clip (DVE 2X_2PORT) + RNE round (Pool) for image i."""
    xp, apadp, wp, tmpp, outp, smallp, psump, psmallp, dramp = pools
    AF = mybir.ActivationFunctionType
    OP = mybir.AluOpType
    a_t = pre["a_t"][it % 2]
    s_q, b_q = st["s_q"], st["b_q"]

    u_sb = tmpp.tile([C, S], fp32, tag="u", name=f"u{it}_{i}")
    nc.scalar.activation(out=u_sb[:], in_=st["xH"][i][:], func=AF.Identity,
                         scale=s_q[:], bias=b_q[:])
    nc.vector.tensor_scalar(u_sb[:], u_sb[:], 3.0, 0.0, OP.min, OP.max)
    # RNE round via MAGIC into the padded fp8 tile, half-image granularity
    for (r0, r1) in ((0, 28), (28, 56)):
        lo, hi = r0 * HW, r1 * HW
        nc.gpsimd.tensor_scalar(a_t[i][:, r0 + 1:r1 + 1, 2:58],
                                u_sb[:, lo:hi].rearrange(
                                    "p (h w) -> p h w", h=r1 - r0),
                                MAGIC, MAGIC, OP.add, OP.subtract)


def _emit_conv_image(nc, pools, params, pre, it, i):
    """3x3 fp8 conv for image i of iteration `it` + drains + out DMA."""
    xp, apadp, wp, tmpp, outp, smallp, psump, psmallp, dramp = pools
    y_d = params[4]
    AF = mybir.ActivationFunctionType
    wq, alpha = pre["wq"], pre["alpha"]
    a_t = pre["a_t"][it % 2]

    out_sb = outp.tile([C, S], fp16, tag="o", name=f"out_sb{it}_{i}")
    base = a_t[i][:]
    ps = [psump.tile([C, NFREE], fp32, tag=f"ps{c}", name=f"ps{it}_{i}_{c}")
          for c in range(NT)]
    for cix in range(NT):
        h0 = cix * R
        for g in range(5):
            if g < 3:
                kw = g
                rhs = bass.AP(
                    tensor=base.tensor,
                    offset=base.offset + h0 * PCW + (kw + 1),
                    ap=[base.ap[0], [PCW, 2], [PCW, R], [1, HW]],
                )
                nc.tensor.matmul(ps[cix][:], lhsT=wq[:, 2 * kw: 2 * kw + 2, :],
                                 rhs=rhs, start=(g == 0), stop=False,
                                 perf_mode=mybir.MatmulPerfMode.DoubleRow)
            elif g == 3:
                rhs = bass.AP(
                    tensor=base.tensor,
                    offset=base.offset + (h0 + 2) * PCW + 1,
                    ap=[base.ap[0], [1, 2], [PCW, R], [1, HW]],
                )
                nc.tensor.matmul(ps[cix][:], lhsT=wq[:, 6:8, :],
                                 rhs=rhs, start=False, stop=False,
                                 perf_mode=mybir.MatmulPerfMode.DoubleRow)
            else:
                rhs = a_t[i][:, h0 + 2: h0 + 2 + R, 3: 3 + HW]
                nc.tensor.matmul(ps[cix][:], lhsT=wq[:, 8, :], rhs=rhs,
                                 start=False, stop=True)
        # scale out of PSUM on ScalarE; fp16 output halves the out stream
        nc.scalar.activation(out=out_sb[:, h0 * HW: (h0 + R) * HW],
                             in_=ps[cix][:], func=AF.Identity,
                             scale=alpha[:])
        if cix % 2 == 1 or cix == NT - 1:
            olo = (cix // 2) * 2 * NFREE if cix % 2 == 1 else cix * NFREE
            ohi = (cix + 1) * NFREE
            nc.scalar.dma_start(out=y_d.ap()[i][:, olo:ohi],
                                in_=out_sb[:, olo:ohi])


def _build(n_iters=1):
    nc = bacc.Bacc("TRN2", target_bir_lowering=False, debug=False,
                   num_devices=N_CORES)

    x_d = nc.declare_dram_parameter("x", [IMG, C, S], fp32, isOutput=False)
    gamma_d = nc.declare_dram_parameter("gamma", [C], fp32, isOutput=False)
    beta_d = nc.declare_dram_parameter("beta", [C], fp32, isOutput=False)
    w_d = nc.declare_dram_parameter("weight", [C, 128 * 9], fp32, isOutput=False)
    y_d = nc.declare_dram_parameter("y", [IMG, C, S], fp16, isOutput=True)
    params = (x_d, gamma_d, beta_d, w_d, y_d)

    with tile.TileContext(nc) as tc:
        with (
            tc.tile_pool(name="xp", bufs=2) as xp,
            tc.tile_pool(name="apad", bufs=1) as apadp,
            tc.tile_pool(name="wp", bufs=1) as wp,
            tc.tile_pool(name="tmp", bufs=4) as tmpp,
            tc.tile_pool(name="outp", bufs=2) as outp,
            tc.tile_pool(name="small", bufs=1) as smallp,
            tc.tile_pool(name="psum", bufs=1, space="PSUM") as psump,
            tc.tile_pool(name="psmall", bufs=1, space="PSUM") as psmallp,
            tc.tile_pool(name="dram", bufs=4, space="DRAM") as dramp,
        ):
            pools = (xp, apadp, wp, tmpp, outp, smallp, psump, psmallp, dramp)
            pre = _emit_prelude(nc, tc, pools, params)
            sts = {}
            for w in range(n_iters + 2):
                fw, pw, cw = w, w - 1, w - 2
                if fw < n_iters:
                    sts[fw] = _emit_front_loads(nc, pools, params, fw)
                if 0 <= pw < n_iters:
                    _emit_prep_chain(nc, pools, pre, sts[pw], pw)
                for m in range(IMG):
                    if 0 <= cw < n_iters:
                        _emit_conv_image(nc, pools, params, pre, cw, m)
                    if 0 <= pw < n_iters:
                        _emit_prep_image(nc, pools, pre, sts[pw], pw, m)
                if fw < n_iters:
                    _emit_front_stats(nc, pools, params, sts[fw], fw)
                if 0 <= cw:
                    sts.pop(cw, None)

    nc.finalize()
    return nc


def _get_nc(n_iters=1):
    key = ("nc", n_iters)
    if key not in _CACHE:
        _CACHE[key] = _build(n_iters)
    return _CACHE[key]


def make_in_maps(x, gamma, beta, weight):
    x = np.ascontiguousarray(np.asarray(x, np.float32)).reshape(N_CORES, IMG, C, S)
    w = np.ascontiguousarray(np.asarray(weight, np.float32)).reshape(C, 128 * 9)
    gamma = np.ascontiguousarray(np.asarray(gamma, np.float32))
    beta = np.ascontiguousarray(np.asarray(beta, np.float32))
    return [
        {"x": x[c], "gamma": gamma, "beta": beta, "weight": w}
        for c in range(N_CORES)
    ]


def kernel(x, gamma, beta, weight):
    import os
    from concourse.bass_utils import run_bass_kernel_spmd

    nc = _get_nc()
    in_maps = make_in_maps(x, gamma, beta, weight)
    core_ids = list(range(N_CORES))
    try:
        res = run_bass_kernel_spmd(nc, in_maps, core_ids)
    except ModuleNotFoundError:
        os.environ["BASS_NEVER_TRACE"] = "1"
        res = run_bass_kernel_spmd(nc, in_maps, core_ids)
    out = np.stack([res.results[c]["y"] for c in range(N_CORES)], axis=0)
    return out.reshape(32, C, HW, HW).astype(np.float32)


# revision 12
# speedup vs baseline: 3.4586x; 3.4586x over previous
"""Trainium2 Bass kernel for BasicQuantConv2d (sync-BN + HWGQ + gauss-quant + 3x3 conv).

Strategy (8 NeuronCores, data-parallel over batch):
  - Each core takes 4 of the 32 images: x shard [4, 128, 56, 56].
  - BN batch stats: per-core bn_stats/bn_aggr -> (mean, E[x^2]) payload,
    AllGather across the 8 cores + local 8-way sum (sync-BN; AllGather is
    ~2x cheaper than AllReduce for a 1KB payload), then the exact /8 is a
    power-of-two scale. Per-channel scale/bias follow.
  - BN + HWGQ folds to ia = RNE_round(clip(x*s_c + b_c, 0, 3)) in {0..3};
    RNE rounding via the fp32 magic constant 1.5*2^23 (matches jnp.round).
  - gauss_quantize(w) == iw * (step/2) with iw in {-3,-1,1,3}; std(w) is
    computed on-device (reduction + ones-matmul broadcast + Newton-refined
    rsqrt), weights transposed per-tap on the PE for the conv lhsT.
  - The 3x3 conv runs in fp8e4m3 (ia in {0..3}, iw in {-3,-1,1,3} are exact
    in fp8; PSUM accumulates fp32 => conv is EXACT integer arithmetic).
    Per output row-chunk: 5 passes -- 3 DoubleRow matmuls (vertical tap
    pairs kh=0&1 per kw, pair-step 64B via the padded row width), 1
    DoubleRow pairing (2,0)+(2,1) horizontally (pair-step 1B), and 1 plain
    fp8 matmul for (2,2) -- accumulated into 7 PSUM banks per image.
  - Output = PSUM * (0.538*step/2) via ScalarE, 896-col DMA out.

Pipelining: `_build(n_iters=K)` emits a prelude (weight path, pad memsets,
gamma/beta) once, then K software-pipelined iterations: each loop round
emits the NEXT iteration's loads+stats (front) before this iteration's
reduce/chain/phase C (back), so the in-order DVE queue processes next-
iteration bn_stats during this iteration's collective. x tiles are double-
buffered (bufs=2), all BN affines run before the per-image clip/round/conv
so x buffers release early, DMA queues are split (x loads on sync, outputs
on scalar, payload/gather on gpsimd), and per-image "bridge" PE fillers keep
the tensor engine's p-state ramp hot across inter-image dependency gaps.
Output DMAs move fp16 (host casts back to fp32); that halves the output
stream at <=2^-11 added relative error. test.py measures per-iteration
device time through the ~80ms axon RPC floor.
"""

import os

import numpy as np

import concourse.bacc as bacc
import concourse.bass as bass
import concourse.tile as tile
from concourse import mybir
from concourse.masks import make_identity

N_CORES = 8
IMG = 4            # images per core
C = 128            # channels (= partitions)
HW = 56
S = HW * HW        # 3136 pixels per image
G = 448            # stats/affine granule (8 image rows)
NT = HW // 8       # 7 granules per image
PR = 58            # padded rows
PCW = 64           # padded row width (interior at cols 2..57; pair-step 64B for DoubleRow)
R = 8              # output rows per matmul tile
NFREE = R * HW     # 448 matmul free dim

HWGQ_STEP = 0.538
GAUSS = 0.996
BN_EPS = 1e-3
MAGIC = float(np.float32(1.5 * 2**23))
NW = 128 * 128 * 9          # weight element count

N_FILL = 14        # PE filler matmuls per iteration (p-state bridge)
N_BRIDGE = 14      # per-image PE fillers bridging inter-image a_t gaps
X_NEWTON = 2       # rsqrt Newton iterations for the BN scale (critical path)

NOPOOL = os.environ.get("KV1_NOPOOL", "0") == "1"
AFFACT = os.environ.get("KV1_AFFACT", "0") == "1"

_CACHE = {}


def _emit_prelude(nc, tc, pools, params):
    """Iteration-invariant work: weight quantization, pad memsets, gamma/beta."""
    fp32 = mybir.dt.float32
    fp8 = mybir.dt.float8e4
    xp, apadp, wp, tmpp, outp, smallp, psump, psmallp, dramp = pools
    x_d, gamma_d, beta_d, w_d, y_d = params
    AF = mybir.ActivationFunctionType
    OP = mybir.AluOpType

    w_sb = wp.tile([C, 128 * 9], fp32)
    nc.sync.dma_start(out=w_sb[:], in_=w_d.ap())

    ident = smallp.tile([C, 128], fp32, tag="ident")
    make_identity(nc, ident[:])

    # one 2KB PSUM bank sliced 4 ways: transpose ping/pong, pg, fillers —
    # disjoint slices so the tile dep tracker doesn't serialize the pipeline
    psm = psmallp.tile([C, 512], fp32, tag="psm", name="psm")

    # transpose each tap: wT[ci, slot, co]; slots pair (kh=0,kw) with (kh=1,kw)
    # adjacently for DoubleRow; kh=2 taps in slots 6..8 ((2,0),(2,1) pair too).
    # slot order: (0,0),(1,0),(0,1),(1,1),(0,2),(1,2),(2,0),(2,1),(2,2)
    SLOT = {(0, 0): 0, (1, 0): 1, (0, 1): 2, (1, 1): 3,
            (0, 2): 4, (1, 2): 5, (2, 0): 6, (2, 1): 7, (2, 2): 8}
    wT = wp.tile([C, 9, 128], fp32)
    w3 = w_sb[:].rearrange("p (ci t) -> p ci t", t=9)
    for t in range(9):
        kh, kw = divmod(t, 3)
        pt = psm[:, (t % 2) * 128:(t % 2) * 128 + 128]
        nc.tensor.transpose(pt, w3[:, :, t], ident[:])
        nc.scalar.copy(out=wT[:, SLOT[(kh, kw)], :], in_=pt)

    # global sum / sumsq of w: ScalarE accum_out row-sums + ones-matmul bcast
    # (scratch shares the uw buffer -- both are prelude-only, used serially)
    uw = wp.tile([C, 9, 128], fp32)
    w2_sb = uw[:].rearrange("p a b -> p (a b)")
    rsums = smallp.tile([C, 2], fp32, tag="rsums")
    nc.scalar.activation(out=w2_sb, in_=w_sb[:], func=AF.Identity,
                         accum_out=rsums[:, 0:1])
    nc.scalar.activation(out=w2_sb, in_=w_sb[:], func=AF.Square,
                         accum_out=rsums[:, 1:2])
    ones = smallp.tile([C, 128], fp32, tag="ones")
    nc.vector.memset(ones[:], 1.0)
    pg = psm[:, 256:384]
    nc.tensor.matmul(pg[:, 0:2], lhsT=ones[:], rhs=rsums[:], start=True, stop=True)
    gs = smallp.tile([C, 2], fp32, tag="gs")
    nc.vector.tensor_copy(gs[:], pg[:, 0:2])

    # wvar = E[w^2] - E[w]^2 ; rw = rsqrt(wvar) Newton-refined
    wmean = smallp.tile([C, 1], fp32, tag="wmean")
    wvar = smallp.tile([C, 1], fp32, tag="wvar")
    nc.vector.tensor_scalar_mul(wmean[:], gs[:, 0:1], 1.0 / NW)
    nc.vector.tensor_scalar_mul(wvar[:], gs[:, 1:2], 1.0 / NW)
    wm2 = smallp.tile([C, 1], fp32, tag="wm2")
    nc.vector.tensor_mul(wm2[:], wmean[:], wmean[:])
    nc.vector.tensor_sub(wvar[:], wvar[:], wm2[:])

    rw = smallp.tile([C, 1], fp32, tag="rw")
    nc.scalar.activation(out=rw[:], in_=wvar[:], func=AF.Sqrt)
    nc.vector.reciprocal(out=rw[:], in_=rw[:])
    tN = smallp.tile([C, 1], fp32, tag="tN")
    for _ in range(2):
        nc.vector.tensor_mul(tN[:], rw[:], rw[:])
        nc.vector.tensor_mul(tN[:], wvar[:], tN[:])
        nc.vector.tensor_scalar(tN[:], tN[:], -0.5, 1.5, OP.mult, OP.add)
        nc.vector.tensor_mul(rw[:], rw[:], tN[:])

    inv_step = smallp.tile([C, 1], fp32, tag="inv_step")
    nc.vector.tensor_scalar_mul(inv_step[:], rw[:], 1.0 / GAUSS)
    # alpha = 0.538 * step/2 = (0.538*0.996/2) * wvar * rw
    alpha = smallp.tile([C, 1], fp32, tag="alpha")
    nc.vector.tensor_mul(alpha[:], wvar[:], rw[:])
    nc.vector.tensor_scalar_mul(alpha[:], alpha[:], HWGQ_STEP * GAUSS / 2.0)

    # quantize transposed weights -> iw in {-3,-1,1,3} (fp8)
    nc.gpsimd.tensor_scalar(uw[:], wT[:], inv_step[:], 0.5, OP.mult, OP.add)
    nc.gpsimd.tensor_scalar(uw[:], uw[:], MAGIC, MAGIC, OP.add, OP.subtract)
    nc.gpsimd.tensor_scalar(uw[:], uw[:], 2.0, -1.0, OP.mult, OP.add)
    wq = wp.tile([C, 9, 128], fp8)
    nc.gpsimd.tensor_scalar(wq[:], uw[:], 3.0, -3.0, OP.min, OP.max)

    # gamma/beta (iteration-invariant)
    gb = smallp.tile([C, 2], fp32, tag="gb")
    gamma_ap = gamma_d.ap().rearrange("(p one) -> p one", one=1)
    beta_ap = beta_d.ap().rearrange("(p one) -> p one", one=1)
    nc.sync.dma_start(out=gb[:, 0:1], in_=gamma_ap)
    nc.sync.dma_start(out=gb[:, 1:2], in_=beta_ap)

    # padded fp8 activation tiles: interior is rewritten every iteration,
    # borders stay zero forever -> memset once here
    a_t = [apadp.tile([C, PR, PCW], fp8, tag=f"a{i}", name=f"a_t{i}")
           for i in range(IMG)]
    for i in range(IMG):
        nc.gpsimd.memset(a_t[i][:], 0.0)

    return dict(ones=ones, wq=wq, alpha=alpha, gb=gb, a_t=a_t, psm=psm)


def _emit_front(nc, tc, pools, params, ablate=()):
    """Loads + BN stats + payload for one iteration (emitted one iteration
    ahead so next-iteration stats fill the DVE queue during this iteration's
    collective)."""
    fp32 = mybir.dt.float32
    xp, apadp, wp, tmpp, outp, smallp, psump, psmallp, dramp = pools
    x_d, gamma_d, beta_d, w_d, y_d = params
    OP = mybir.AluOpType

    # ---------------- load x (896-col tiles, 448-col granule views) --------
    # x loads ride the sync (SP) queue -- a dedicated issuer that is never
    # blocked by compute and holds nothing but loads, so iteration i+1's
    # loads dispatch as soon as their (double-buffered) tiles free up
    xH = [[xp.tile([C, 896 if h < 3 else G], fp32, tag=f"x{i}_{h}",
                   name=f"x{i}_{h}") for h in range(4)] for i in range(IMG)]
    for i in range(IMG):
        for h in range(4):
            lo, hi = h * 896, min((h + 1) * 896, S)
            nc.sync.dma_start(out=xH[i][h][:], in_=x_d.ap()[i][:, lo:hi])

    def xgran(i, g):
        t = xH[i][g // 2]
        if g % 2 == 0:
            return t[:, 0:G]
        return t[:, G:2 * G]

    stats = smallp.tile([C, IMG * NT, 6], fp32)
    for i in range(IMG):
        for g in range(NT):
            nc.vector.bn_stats(out=stats[:, i * NT + g, :], in_=xgran(i, g))
    # payload: (mean, E[x^2]) raw; E[x^2] = var + mean^2 (in-place on pay)
    pay = smallp.tile([C, 2], fp32)
    nc.vector.bn_aggr(out=pay[:], in_=stats[:])
    m2 = smallp.tile([C, 1], fp32)
    nc.vector.tensor_mul(m2[:], pay[:, 0:1], pay[:, 0:1])
    nc.vector.tensor_add(pay[:, 1:2], pay[:, 1:2], m2[:])

    return dict(xH=xH, xgran=xgran, pay=pay)


def _emit_dispatch(nc, pools, st, ablate=()):
    """Collective dispatch (gpsimd queue): payload out, AllGather, gather-in."""
    fp32 = mybir.dt.float32
    xp, apadp, wp, tmpp, outp, smallp, psump, psmallp, dramp = pools
    OP = mybir.AluOpType
    pay = st["pay"]
    # ---------------- sync-BN cross-core exchange ----------------
    cc_in = dramp.tile([C, 2], fp32)
    cc_gath = dramp.tile([N_CORES, C, 2], fp32)
    nc.gpsimd.dma_start(out=cc_in[:], in_=pay[:])
    if "noar" in ablate:
        for r in range(N_CORES):
            nc.gpsimd.dma_start(out=cc_gath[r], in_=cc_in[:])
    else:
        nc.gpsimd.collective_compute(
            "AllGather",
            OP.bypass,
            replica_groups=[list(range(N_CORES))],
            ins=[cc_in.opt()],
            outs=[cc_gath.opt()],
        )
    g_all = smallp.tile([C, N_CORES, 2], fp32)
    nc.gpsimd.dma_start(out=g_all[:], in_=cc_gath[:].rearrange("r p t -> p r t"))
    st["g_all"] = g_all


def _emit_back(nc, tc, pools, params, pre, st, pipelined=True, ablate=()):
    """Reduce + scale/bias chain + per-image quantize/conv/out."""
    fp32 = mybir.dt.float32
    xp, apadp, wp, tmpp, outp, smallp, psump, psmallp, dramp = pools
    x_d, gamma_d, beta_d, w_d, y_d = params
    AF = mybir.ActivationFunctionType
    OP = mybir.AluOpType
    ones, wq, alpha, gb, a_t = (pre["ones"], pre["wq"], pre["alpha"],
                                pre["gb"], pre["a_t"])
    xH, xgran, g_all = st["xH"], st["xgran"], st["g_all"]

    # local 8-way sum (same order on all cores), then exact /8
    g_sum = smallp.tile([C, 2], fp32)
    nc.vector.tensor_reduce(out=g_sum[:], in_=g_all[:].rearrange("p r t -> p t r"),
                            axis=mybir.AxisListType.X, op=OP.add)

    # PE fillers keep the p-state ramp hot through the collective window;
    # rhs depends on this iteration's first x tile so the scheduler cannot
    # hoist them into the previous iteration's conv burst
    ps_fill = pre["psm"][:, 384:512]
    if "nowarm" not in ablate:
        for _ in range(N_FILL):
            nc.tensor.matmul(ps_fill, lhsT=ones[:], rhs=xH[0][0][:, 0:128],
                             start=True, stop=True)

    # ---------------- global scale/bias ----------------
    # fused but bitwise-identical to the reference chain:
    # me = g_sum*0.125 (exact), vge = (E[x^2] - mean^2) + eps
    me = smallp.tile([C, 2], fp32)      # (global mean, global E[x^2])
    nc.vector.tensor_scalar_mul(me[:], g_sum[:], 0.125)
    meanv = me[:, 0:1]
    vge = smallp.tile([C, 1], fp32)     # var + eps
    gm2 = smallp.tile([C, 1], fp32)
    nc.vector.tensor_mul(gm2[:], meanv, meanv)
    nc.vector.tensor_scalar(vge[:], me[:, 1:2], gm2[:], BN_EPS,
                            OP.subtract, OP.add)
    rx = smallp.tile([C, 1], fp32)
    nc.scalar.activation(out=rx[:], in_=vge[:], func=AF.Sqrt)
    nc.vector.reciprocal(out=rx[:], in_=rx[:])
    tX = smallp.tile([C, 1], fp32)
    for _ in range(X_NEWTON):
        nc.vector.tensor_mul(tX[:], rx[:], rx[:])
        nc.vector.tensor_mul(tX[:], vge[:], tX[:])
        nc.vector.tensor_scalar(tX[:], tX[:], -0.5, 1.5, OP.mult, OP.add)
        nc.vector.tensor_mul(rx[:], rx[:], tX[:])

    # s = gamma * rsqrt / 0.538 ; b = (beta - mean*gamma*rsqrt) / 0.538
    s_q = smallp.tile([C, 1], fp32)
    b_q = smallp.tile([C, 1], fp32)
    ta = smallp.tile([C, 1], fp32)
    nc.vector.tensor_mul(ta[:], gb[:, 0:1], rx[:])          # A = gamma*inv
    nc.vector.tensor_scalar_mul(s_q[:], ta[:], 1.0 / HWGQ_STEP)
    tb = smallp.tile([C, 1], fp32)
    nc.vector.tensor_mul(tb[:], meanv, ta[:])               # mean*A
    nc.vector.tensor_scalar(b_q[:], gb[:, 1:2], tb[:], 1.0 / HWGQ_STEP,
                            OP.subtract, OP.mult)           # (beta-mean*A)/0.538

    # ---------------- per-image quantize + conv ----------------
    # all 28 affines first: x buffers release after ~10us instead of ~17us,
    # which is the binding recurrence for cross-iteration pipelining.
    # Early granules (g<3) on DVE (fast, feeds round chunk 0 quickly),
    # the rest on Pool.
    u_im = []
    for i in range(IMG):
        u_sb = tmpp.tile([C, S], fp32, tag="u", name=f"u_sb{i}")
        u_im.append(u_sb)
        for h in range(4):
            lo, hi = h * 896, min((h + 1) * 896, S)
            if AFFACT:
                nc.scalar.activation(out=u_sb[:, lo:hi], in_=xH[i][h][:],
                                     func=AF.Identity, scale=s_q[:],
                                     bias=b_q[:])
            else:
                nc.vector.tensor_scalar(u_sb[:, lo:hi], xH[i][h][:],
                                        s_q[:], b_q[:], OP.mult, OP.add)
    for i in range(IMG):
        u_sb = u_im[i]
        # clip in place on Pool, then RNE round via MAGIC into the padded
        # fp8 tile (DVE; the last image's rounds go to Pool to shorten the
        # DVE tail, which is the binding engine in steady state)
        for (r0, r1) in ((0, 16), (16, 32), (32, 48), (48, 56)):
            lo, hi = r0 * HW, r1 * HW
            ceng = nc.vector if NOPOOL else nc.gpsimd
            ceng.tensor_scalar(u_sb[:, lo:hi], u_sb[:, lo:hi], 3.0, 0.0,
                               OP.min, OP.max)
            reng = (nc.vector if NOPOOL else
                    (nc.vector if (i < 2 or (i == 2 and r0 < 32))
                     else nc.gpsimd))
            reng.tensor_scalar(a_t[i][:, r0 + 1:r1 + 1, 2:58],
                               u_sb[:, lo:hi].rearrange(
                                   "p (h w) -> p h w", h=r1 - r0),
                               MAGIC, MAGIC, OP.add, OP.subtract)

        # bridge fillers: keep the PE p-state ramp alive across the
        # inter-image a_t dependency gap (dep on this image's clipped u);
        # only useful when iterations pipeline
        for _ in range(N_BRIDGE if pipelined else 0):
            nc.tensor.matmul(pre["psm"][:, 384:512], lhsT=ones[:],
                             rhs=u_sb[:, 0:128], start=True, stop=True)
        out_sb = outp.tile([C, S], mybir.dt.float16, tag="o", name=f"out_sb{i}")
        base = a_t[i][:]
        ps = [psump.tile([C, NFREE], fp32, tag=f"ps{c}", name=f"ps{i}_{c}")
              for c in range(NT)]
        # 5 passes per chunk (cix outer so each PSUM bank completes ASAP):
        # 3 DoubleRow (kh=0&1 per kw), DoubleRow (2,0)+(2,1), single (2,2)
        for cix in range(NT):
            h0 = cix * R
            if "noconv" in ablate:
                continue
            for g in range(5):
                if g < 3:
                    kw = g
                    rhs = bass.AP(
                        tensor=base.tensor,
                        offset=base.offset + h0 * PCW + (kw + 1),
                        ap=[base.ap[0], [PCW, 2], [PCW, R], [1, HW]],
                    )
                    nc.tensor.matmul(ps[cix][:], lhsT=wq[:, 2 * kw: 2 * kw + 2, :],
                                     rhs=rhs, start=(g == 0), stop=False,
                                     perf_mode=mybir.MatmulPerfMode.DoubleRow)
                elif g == 3:
                    rhs = bass.AP(
                        tensor=base.tensor,
                        offset=base.offset + (h0 + 2) * PCW + 1,
                        ap=[base.ap[0], [1, 2], [PCW, R], [1, HW]],
                    )
                    nc.tensor.matmul(ps[cix][:], lhsT=wq[:, 6:8, :],
                                     rhs=rhs, start=False, stop=False,
                                     perf_mode=mybir.MatmulPerfMode.DoubleRow)
                else:
                    rhs = a_t[i][:, h0 + 2: h0 + 2 + R, 3: 3 + HW]
                    nc.tensor.matmul(ps[cix][:], lhsT=wq[:, 8, :], rhs=rhs,
                                     start=False, stop=True)
            # scale out of PSUM on ScalarE (gpsimd cannot read PSUM);
            # fp16 output halves the out-DMA stream (adds <=2^-11 relative
            # rounding, far inside the error budget)
            nc.scalar.activation(out=out_sb[:, h0 * HW: (h0 + R) * HW],
                                 in_=ps[cix][:], func=AF.Identity,
                                 scale=alpha[:])
            # 896-col output DMAs (sync queue) to halve descriptor count
            if cix % 2 == 1 or cix == NT - 1:
                olo = (cix // 2) * 2 * NFREE if cix % 2 == 1 else cix * NFREE
                ohi = (cix + 1) * NFREE
                nc.scalar.dma_start(out=y_d.ap()[i][:, olo:ohi],
                                      in_=out_sb[:, olo:ohi])


def _build(n_iters=1, ablate=()):
    fp32 = mybir.dt.float32

    nc = bacc.Bacc("TRN2", target_bir_lowering=False, debug=False,
                   num_devices=N_CORES)

    x_d = nc.declare_dram_parameter("x", [IMG, C, S], fp32, isOutput=False)
    gamma_d = nc.declare_dram_parameter("gamma", [C], fp32, isOutput=False)
    beta_d = nc.declare_dram_parameter("beta", [C], fp32, isOutput=False)
    w_d = nc.declare_dram_parameter("weight", [C, 128 * 9], fp32, isOutput=False)
    y_d = nc.declare_dram_parameter("y", [IMG, C, S], mybir.dt.float16,
                                    isOutput=True)
    params = (x_d, gamma_d, beta_d, w_d, y_d)

    with tile.TileContext(nc) as tc:
        with (
            tc.tile_pool(name="xp", bufs=2) as xp,
            tc.tile_pool(name="apad", bufs=1) as apadp,
            tc.tile_pool(name="wp", bufs=1) as wp,
            tc.tile_pool(name="tmp", bufs=4) as tmpp,
            tc.tile_pool(name="outp", bufs=2) as outp,
            tc.tile_pool(name="small", bufs=1) as smallp,
            tc.tile_pool(name="psum", bufs=1, space="PSUM") as psump,
            tc.tile_pool(name="psmall", bufs=1, space="PSUM") as psmallp,
            tc.tile_pool(name="dram", bufs=4, space="DRAM") as dramp,
        ):
            pools = (xp, apadp, wp, tmpp, outp, smallp, psump, psmallp, dramp)
            pre = _emit_prelude(nc, tc, pools, params)
            st = _emit_front(nc, tc, pools, params, ablate)
            _emit_dispatch(nc, pools, st, ablate)
            for it in range(n_iters):
                nst = (_emit_front(nc, tc, pools, params, ablate)
                       if it + 1 < n_iters else None)
                _emit_back(nc, tc, pools, params, pre, st,
                           pipelined=n_iters > 1, ablate=ablate)
                if nst is not None:
                    _emit_dispatch(nc, pools, nst, ablate)
                st = nst

    nc.finalize()
    return nc


def _get_nc(n_iters=1):
    key = ("nc", n_iters)
    if key not in _CACHE:
        _CACHE[key] = _build(n_iters)
    return _CACHE[key]


def make_in_maps(x, gamma, beta, weight):
    x = np.ascontiguousarray(np.asarray(x, np.float32)).reshape(N_CORES, IMG, C, S)
    w = np.ascontiguousarray(np.asarray(weight, np.float32)).reshape(C, 128 * 9)
    gamma = np.ascontiguousarray(np.asarray(gamma, np.float32))
    beta = np.ascontiguousarray(np.asarray(beta, np.float32))
    return [
        {"x": x[c], "gamma": gamma, "beta": beta, "weight": w}
        for c in range(N_CORES)
    ]


def kernel(x, gamma, beta, weight):
    import os
    from concourse.bass_utils import run_bass_kernel_spmd

    nc = _get_nc()
    in_maps = make_in_maps(x, gamma, beta, weight)
    core_ids = list(range(N_CORES))
    try:
        res = run_bass_kernel_spmd(nc, in_maps, core_ids)
    except ModuleNotFoundError:
        # BASS_TRACE set but no NTFF profile hook in this container
        os.environ["BASS_NEVER_TRACE"] = "1"
        res = run_bass_kernel_spmd(nc, in_maps, core_ids)
    out = np.stack([res.results[c]["y"] for c in range(N_CORES)], axis=0)
    return out.reshape(32, C, HW, HW).astype(np.float32)



# revision 13
# speedup vs baseline: 3.8164x; 1.1035x over previous
"""Trainium2 Bass kernel for BasicQuantConv2d (sync-BN + HWGQ + gauss-quant + 3x3 conv).

Strategy (8 NeuronCores, data-parallel over batch):
  - Each core takes 4 of the 32 images: x shard [4, 128, 56, 56].
  - BN batch stats: per-core bn_stats/bn_aggr -> (mean, E[x^2]) payload,
    AllGather across the 8 cores + local 8-way sum (sync-BN; AllGather is
    ~2x cheaper than AllReduce for a 1KB payload), then the exact /8 is a
    power-of-two scale. Per-channel scale/bias follow.
  - BN + HWGQ folds to ia = RNE_round(clip(x*s_c + b_c, 0, 3)) in {0..3};
    RNE rounding via the fp32 magic constant 1.5*2^23 (matches jnp.round).
  - gauss_quantize(w) == iw * (step/2) with iw in {-3,-1,1,3}; std(w) is
    computed on-device (reduction + ones-matmul broadcast + Newton-refined
    rsqrt), weights transposed per-tap on the PE for the conv lhsT.
  - The 3x3 conv runs in fp8e4m3 (ia in {0..3}, iw in {-3,-1,1,3} are exact
    in fp8; PSUM accumulates fp32 => conv is EXACT integer arithmetic).
    Per output row-chunk: 5 passes -- 3 DoubleRow matmuls (vertical tap
    pairs kh=0&1 per kw, pair-step 64B via the padded row width), 1
    DoubleRow pairing (2,0)+(2,1) horizontally (pair-step 1B), and 1 plain
    fp8 matmul for (2,2) -- accumulated into 7 PSUM banks per image.
  - Output = PSUM * (0.538*step/2) via ScalarE, 896-col DMA out.

Pipelining: `_build(n_iters=K)` emits a prelude (weight path, pad memsets,
gamma/beta) once, then K software-pipelined iterations: each loop round
emits the NEXT iteration's loads+stats (front) before this iteration's
reduce/chain/phase C (back), so the in-order DVE queue processes next-
iteration bn_stats during this iteration's collective. x tiles are double-
buffered (bufs=2), all BN affines run before the per-image clip/round/conv
so x buffers release early, DMA queues are split (x loads on sync, outputs
on scalar, payload/gather on gpsimd), and per-image "bridge" PE fillers keep
the tensor engine's p-state ramp hot across inter-image dependency gaps.
Output DMAs move fp16 (host casts back to fp32); that halves the output
stream at <=2^-11 added relative error. test.py measures per-iteration
device time through the ~80ms axon RPC floor.
"""

import os

import numpy as np

import concourse.bacc as bacc
import concourse.bass as bass
import concourse.tile as tile
from concourse import mybir
from concourse.masks import make_identity

N_CORES = 8
IMG = 4            # images per core
C = 128            # channels (= partitions)
HW = 56
S = HW * HW        # 3136 pixels per image
G = 448            # stats/affine granule (8 image rows)
NT = HW // 8       # 7 granules per image
PR = 58            # padded rows
PCW = 64           # padded row width (interior at cols 2..57; pair-step 64B for DoubleRow)
R = 8              # output rows per matmul tile
NFREE = R * HW     # 448 matmul free dim

HWGQ_STEP = 0.538
GAUSS = 0.996
BN_EPS = 1e-3
MAGIC = float(np.float32(1.5 * 2**23))
NW = 128 * 128 * 9          # weight element count

N_FILL = 14        # PE filler matmuls per iteration (p-state bridge)
N_BRIDGE = 14      # per-image PE fillers bridging inter-image a_t gaps
X_NEWTON = 2       # rsqrt Newton iterations for the BN scale (critical path)

NOPOOL = os.environ.get("KV1_NOPOOL", "1") == "1"
AFFACT = os.environ.get("KV1_AFFACT", "0") == "1"

_CACHE = {}


def _emit_prelude(nc, tc, pools, params):
    """Iteration-invariant work: weight quantization, pad memsets, gamma/beta."""
    fp32 = mybir.dt.float32
    fp8 = mybir.dt.float8e4
    xp, apadp, wp, tmpp, outp, smallp, psump, psmallp, dramp = pools
    x_d, gamma_d, beta_d, w_d, y_d = params
    AF = mybir.ActivationFunctionType
    OP = mybir.AluOpType

    w_sb = wp.tile([C, 128 * 9], fp32)
    nc.sync.dma_start(out=w_sb[:], in_=w_d.ap())

    ident = smallp.tile([C, 128], fp32, tag="ident")
    make_identity(nc, ident[:])

    # one 2KB PSUM bank sliced 4 ways: transpose ping/pong, pg, fillers —
    # disjoint slices so the tile dep tracker doesn't serialize the pipeline
    psm = psmallp.tile([C, 512], fp32, tag="psm", name="psm")

    # transpose each tap: wT[ci, slot, co]; slots pair (kh=0,kw) with (kh=1,kw)
    # adjacently for DoubleRow; kh=2 taps in slots 6..8 ((2,0),(2,1) pair too).
    # slot order: (0,0),(1,0),(0,1),(1,1),(0,2),(1,2),(2,0),(2,1),(2,2)
    SLOT = {(0, 0): 0, (1, 0): 1, (0, 1): 2, (1, 1): 3,
            (0, 2): 4, (1, 2): 5, (2, 0): 6, (2, 1): 7, (2, 2): 8}
    wT = wp.tile([C, 9, 128], fp32)
    w3 = w_sb[:].rearrange("p (ci t) -> p ci t", t=9)
    for t in range(9):
        kh, kw = divmod(t, 3)
        pt = psm[:, (t % 2) * 128:(t % 2) * 128 + 128]
        nc.tensor.transpose(pt, w3[:, :, t], ident[:])
        nc.scalar.copy(out=wT[:, SLOT[(kh, kw)], :], in_=pt)

    # global sum / sumsq of w: ScalarE accum_out row-sums + ones-matmul bcast
    # (scratch shares the uw buffer -- both are prelude-only, used serially)
    uw = wp.tile([C, 9, 128], fp32)
    w2_sb = uw[:].rearrange("p a b -> p (a b)")
    rsums = smallp.tile([C, 2], fp32, tag="rsums")
    nc.scalar.activation(out=w2_sb, in_=w_sb[:], func=AF.Identity,
                         accum_out=rsums[:, 0:1])
    nc.scalar.activation(out=w2_sb, in_=w_sb[:], func=AF.Square,
                         accum_out=rsums[:, 1:2])
    ones = smallp.tile([C, 128], fp32, tag="ones")
    nc.vector.memset(ones[:], 1.0)
    pg = psm[:, 256:384]
    nc.tensor.matmul(pg[:, 0:2], lhsT=ones[:], rhs=rsums[:], start=True, stop=True)
    gs = smallp.tile([C, 2], fp32, tag="gs")
    nc.vector.tensor_copy(gs[:], pg[:, 0:2])

    # wvar = E[w^2] - E[w]^2 ; rw = rsqrt(wvar) Newton-refined
    wmean = smallp.tile([C, 1], fp32, tag="wmean")
    wvar = smallp.tile([C, 1], fp32, tag="wvar")
    nc.vector.tensor_scalar_mul(wmean[:], gs[:, 0:1], 1.0 / NW)
    nc.vector.tensor_scalar_mul(wvar[:], gs[:, 1:2], 1.0 / NW)
    wm2 = smallp.tile([C, 1], fp32, tag="wm2")
    nc.vector.tensor_mul(wm2[:], wmean[:], wmean[:])
    nc.vector.tensor_sub(wvar[:], wvar[:], wm2[:])

    rw = smallp.tile([C, 1], fp32, tag="rw")
    nc.scalar.activation(out=rw[:], in_=wvar[:], func=AF.Sqrt)
    nc.vector.reciprocal(out=rw[:], in_=rw[:])
    tN = smallp.tile([C, 1], fp32, tag="tN")
    for _ in range(2):
        nc.vector.tensor_mul(tN[:], rw[:], rw[:])
        nc.vector.tensor_mul(tN[:], wvar[:], tN[:])
        nc.vector.tensor_scalar(tN[:], tN[:], -0.5, 1.5, OP.mult, OP.add)
        nc.vector.tensor_mul(rw[:], rw[:], tN[:])

    inv_step = smallp.tile([C, 1], fp32, tag="inv_step")
    nc.vector.tensor_scalar_mul(inv_step[:], rw[:], 1.0 / GAUSS)
    # alpha = 0.538 * step/2 = (0.538*0.996/2) * wvar * rw
    alpha = smallp.tile([C, 1], fp32, tag="alpha")
    nc.vector.tensor_mul(alpha[:], wvar[:], rw[:])
    nc.vector.tensor_scalar_mul(alpha[:], alpha[:], HWGQ_STEP * GAUSS / 2.0)

    # quantize transposed weights -> iw in {-3,-1,1,3} (fp8)
    nc.gpsimd.tensor_scalar(uw[:], wT[:], inv_step[:], 0.5, OP.mult, OP.add)
    nc.gpsimd.tensor_scalar(uw[:], uw[:], MAGIC, MAGIC, OP.add, OP.subtract)
    nc.gpsimd.tensor_scalar(uw[:], uw[:], 2.0, -1.0, OP.mult, OP.add)
    wq = wp.tile([C, 9, 128], fp8)
    nc.gpsimd.tensor_scalar(wq[:], uw[:], 3.0, -3.0, OP.min, OP.max)

    # gamma/beta (iteration-invariant)
    gb = smallp.tile([C, 2], fp32, tag="gb")
    gamma_ap = gamma_d.ap().rearrange("(p one) -> p one", one=1)
    beta_ap = beta_d.ap().rearrange("(p one) -> p one", one=1)
    nc.sync.dma_start(out=gb[:, 0:1], in_=gamma_ap)
    nc.sync.dma_start(out=gb[:, 1:2], in_=beta_ap)

    # padded fp8 activation tiles: interior is rewritten every iteration,
    # borders stay zero forever -> memset once here
    a_t = [apadp.tile([C, PR, PCW], fp8, tag=f"a{i}", name=f"a_t{i}")
           for i in range(IMG)]
    for i in range(IMG):
        nc.gpsimd.memset(a_t[i][:], 0.0)

    return dict(ones=ones, wq=wq, alpha=alpha, gb=gb, a_t=a_t, psm=psm)


def _emit_front(nc, tc, pools, params, ablate=()):
    """Loads + BN stats + payload for one iteration (emitted one iteration
    ahead so next-iteration stats fill the DVE queue during this iteration's
    collective)."""
    fp32 = mybir.dt.float32
    xp, apadp, wp, tmpp, outp, smallp, psump, psmallp, dramp = pools
    x_d, gamma_d, beta_d, w_d, y_d = params
    OP = mybir.AluOpType

    # ---------------- load x (896-col tiles, 448-col granule views) --------
    # x loads ride the sync (SP) queue -- a dedicated issuer that is never
    # blocked by compute and holds nothing but loads, so iteration i+1's
    # loads dispatch as soon as their (double-buffered) tiles free up
    xH = [[xp.tile([C, 896 if h < 3 else G], fp32, tag=f"x{i}_{h}",
                   name=f"x{i}_{h}") for h in range(4)] for i in range(IMG)]
    for i in range(IMG):
        for h in range(4):
            lo, hi = h * 896, min((h + 1) * 896, S)
            nc.sync.dma_start(out=xH[i][h][:], in_=x_d.ap()[i][:, lo:hi])

    def xgran(i, g):
        t = xH[i][g // 2]
        if g % 2 == 0:
            return t[:, 0:G]
        return t[:, G:2 * G]

    stats = smallp.tile([C, IMG * NT, 6], fp32)
    for i in range(IMG):
        for g in range(NT):
            nc.vector.bn_stats(out=stats[:, i * NT + g, :], in_=xgran(i, g))
    # payload: (mean, E[x^2]) raw; E[x^2] = var + mean^2 (in-place on pay)
    pay = smallp.tile([C, 2], fp32)
    nc.vector.bn_aggr(out=pay[:], in_=stats[:])
    m2 = smallp.tile([C, 1], fp32)
    nc.vector.tensor_mul(m2[:], pay[:, 0:1], pay[:, 0:1])
    nc.vector.tensor_add(pay[:, 1:2], pay[:, 1:2], m2[:])

    return dict(xH=xH, xgran=xgran, pay=pay)


def _emit_dispatch(nc, pools, st, ablate=()):
    """Collective dispatch (gpsimd queue): payload out, AllGather, gather-in."""
    fp32 = mybir.dt.float32
    xp, apadp, wp, tmpp, outp, smallp, psump, psmallp, dramp = pools
    OP = mybir.AluOpType
    pay = st["pay"]
    # ---------------- sync-BN cross-core exchange ----------------
    cc_in = dramp.tile([C, 2], fp32)
    cc_gath = dramp.tile([N_CORES, C, 2], fp32)
    nc.gpsimd.dma_start(out=cc_in[:], in_=pay[:])
    if "noar" in ablate:
        for r in range(N_CORES):
            nc.gpsimd.dma_start(out=cc_gath[r], in_=cc_in[:])
    else:
        nc.gpsimd.collective_compute(
            "AllGather",
            OP.bypass,
            replica_groups=[list(range(N_CORES))],
            ins=[cc_in.opt()],
            outs=[cc_gath.opt()],
        )
    g_all = smallp.tile([C, N_CORES, 2], fp32)
    nc.gpsimd.dma_start(out=g_all[:], in_=cc_gath[:].rearrange("r p t -> p r t"))
    st["g_all"] = g_all


def _emit_back(nc, tc, pools, params, pre, st, pipelined=True, ablate=()):
    """Reduce + scale/bias chain + per-image quantize/conv/out."""
    fp32 = mybir.dt.float32
    xp, apadp, wp, tmpp, outp, smallp, psump, psmallp, dramp = pools
    x_d, gamma_d, beta_d, w_d, y_d = params
    AF = mybir.ActivationFunctionType
    OP = mybir.AluOpType
    ones, wq, alpha, gb, a_t = (pre["ones"], pre["wq"], pre["alpha"],
                                pre["gb"], pre["a_t"])
    xH, xgran, g_all = st["xH"], st["xgran"], st["g_all"]

    # local 8-way sum (same order on all cores), then exact /8
    g_sum = smallp.tile([C, 2], fp32)
    nc.vector.tensor_reduce(out=g_sum[:], in_=g_all[:].rearrange("p r t -> p t r"),
                            axis=mybir.AxisListType.X, op=OP.add)

    # PE fillers keep the p-state ramp hot through the collective window;
    # rhs depends on this iteration's first x tile so the scheduler cannot
    # hoist them into the previous iteration's conv burst
    ps_fill = pre["psm"][:, 384:512]
    if "nowarm" not in ablate:
        for _ in range(N_FILL):
            nc.tensor.matmul(ps_fill, lhsT=ones[:], rhs=xH[0][0][:, 0:128],
                             start=True, stop=True)

    # ---------------- global scale/bias ----------------
    # fused but bitwise-identical to the reference chain:
    # me = g_sum*0.125 (exact), vge = (E[x^2] - mean^2) + eps
    me = smallp.tile([C, 2], fp32)      # (global mean, global E[x^2])
    nc.vector.tensor_scalar_mul(me[:], g_sum[:], 0.125)
    meanv = me[:, 0:1]
    vge = smallp.tile([C, 1], fp32)     # var + eps
    gm2 = smallp.tile([C, 1], fp32)
    nc.vector.tensor_mul(gm2[:], meanv, meanv)
    nc.vector.tensor_scalar(vge[:], me[:, 1:2], gm2[:], BN_EPS,
                            OP.subtract, OP.add)
    rx = smallp.tile([C, 1], fp32)
    nc.scalar.activation(out=rx[:], in_=vge[:], func=AF.Sqrt)
    nc.vector.reciprocal(out=rx[:], in_=rx[:])
    tX = smallp.tile([C, 1], fp32)
    for _ in range(X_NEWTON):
        nc.vector.tensor_mul(tX[:], rx[:], rx[:])
        nc.vector.tensor_mul(tX[:], vge[:], tX[:])
        nc.vector.tensor_scalar(tX[:], tX[:], -0.5, 1.5, OP.mult, OP.add)
        nc.vector.tensor_mul(rx[:], rx[:], tX[:])

    # s = gamma * rsqrt / 0.538 ; b = (beta - mean*gamma*rsqrt) / 0.538
    s_q = smallp.tile([C, 1], fp32)
    b_q = smallp.tile([C, 1], fp32)
    ta = smallp.tile([C, 1], fp32)
    nc.vector.tensor_mul(ta[:], gb[:, 0:1], rx[:])          # A = gamma*inv
    nc.vector.tensor_scalar_mul(s_q[:], ta[:], 1.0 / HWGQ_STEP)
    tb = smallp.tile([C, 1], fp32)
    nc.vector.tensor_mul(tb[:], meanv, ta[:])               # mean*A
    nc.vector.tensor_scalar(b_q[:], gb[:, 1:2], tb[:], 1.0 / HWGQ_STEP,
                            OP.subtract, OP.mult)           # (beta-mean*A)/0.538

    # ---------------- per-image quantize + conv ----------------
    # all 28 affines first: x buffers release after ~10us instead of ~17us,
    # which is the binding recurrence for cross-iteration pipelining.
    # Early granules (g<3) on DVE (fast, feeds round chunk 0 quickly),
    # the rest on Pool.
    u_im = []
    for i in range(IMG):
        u_sb = tmpp.tile([C, S], fp32, tag="u", name=f"u_sb{i}")
        u_im.append(u_sb)
        for h in range(4):
            lo, hi = h * 896, min((h + 1) * 896, S)
            if AFFACT:
                nc.scalar.activation(out=u_sb[:, lo:hi], in_=xH[i][h][:],
                                     func=AF.Identity, scale=s_q[:],
                                     bias=b_q[:])
            else:
                nc.vector.tensor_scalar(u_sb[:, lo:hi], xH[i][h][:],
                                        s_q[:], b_q[:], OP.mult, OP.add)
    for i in range(IMG):
        u_sb = u_im[i]
        # clip in place on Pool, then RNE round via MAGIC into the padded
        # fp8 tile (DVE; the last image's rounds go to Pool to shorten the
        # DVE tail, which is the binding engine in steady state)
        for (r0, r1) in ((0, 16), (16, 32), (32, 48), (48, 56)):
            lo, hi = r0 * HW, r1 * HW
            ceng = nc.vector if NOPOOL else nc.gpsimd
            ceng.tensor_scalar(u_sb[:, lo:hi], u_sb[:, lo:hi], 3.0, 0.0,
                               OP.min, OP.max)
            reng = (nc.vector if NOPOOL else
                    (nc.vector if (i < 2 or (i == 2 and r0 < 32))
                     else nc.gpsimd))
            reng.tensor_scalar(a_t[i][:, r0 + 1:r1 + 1, 2:58],
                               u_sb[:, lo:hi].rearrange(
                                   "p (h w) -> p h w", h=r1 - r0),
                               MAGIC, MAGIC, OP.add, OP.subtract)

        # bridge fillers: keep the PE p-state ramp alive across the
        # inter-image a_t dependency gap (dep on this image's clipped u);
        # only useful when iterations pipeline
        for _ in range(N_BRIDGE if pipelined else 0):
            nc.tensor.matmul(pre["psm"][:, 384:512], lhsT=ones[:],
                             rhs=u_sb[:, 0:128], start=True, stop=True)
        out_sb = outp.tile([C, S], mybir.dt.float16, tag="o", name=f"out_sb{i}")
        base = a_t[i][:]
        ps = [psump.tile([C, NFREE], fp32, tag=f"ps{c}", name=f"ps{i}_{c}")
              for c in range(NT)]
        # 5 passes per chunk (cix outer so each PSUM bank completes ASAP):
        # 3 DoubleRow (kh=0&1 per kw), DoubleRow (2,0)+(2,1), single (2,2)
        for cix in range(NT):
            h0 = cix * R
            if "noconv" in ablate:
                continue
            for g in range(5):
                if g < 3:
                    kw = g
                    rhs = bass.AP(
                        tensor=base.tensor,
                        offset=base.offset + h0 * PCW + (kw + 1),
                        ap=[base.ap[0], [PCW, 2], [PCW, R], [1, HW]],
                    )
                    nc.tensor.matmul(ps[cix][:], lhsT=wq[:, 2 * kw: 2 * kw + 2, :],
                                     rhs=rhs, start=(g == 0), stop=False,
                                     perf_mode=mybir.MatmulPerfMode.DoubleRow)
                elif g == 3:
                    rhs = bass.AP(
                        tensor=base.tensor,
                        offset=base.offset + (h0 + 2) * PCW + 1,
                        ap=[base.ap[0], [1, 2], [PCW, R], [1, HW]],
                    )
                    nc.tensor.matmul(ps[cix][:], lhsT=wq[:, 6:8, :],
                                     rhs=rhs, start=False, stop=False,
                                     perf_mode=mybir.MatmulPerfMode.DoubleRow)
                else:
                    rhs = a_t[i][:, h0 + 2: h0 + 2 + R, 3: 3 + HW]
                    nc.tensor.matmul(ps[cix][:], lhsT=wq[:, 8, :], rhs=rhs,
                                     start=False, stop=True)
            # scale out of PSUM on ScalarE (gpsimd cannot read PSUM);
            # fp16 output halves the out-DMA stream (adds <=2^-11 relative
            # rounding, far inside the error budget)
            nc.scalar.activation(out=out_sb[:, h0 * HW: (h0 + R) * HW],
                                 in_=ps[cix][:], func=AF.Identity,
                                 scale=alpha[:])
            # 896-col output DMAs (sync queue) to halve descriptor count
            if cix % 2 == 1 or cix == NT - 1:
                olo = (cix // 2) * 2 * NFREE if cix % 2 == 1 else cix * NFREE
                ohi = (cix + 1) * NFREE
                nc.scalar.dma_start(out=y_d.ap()[i][:, olo:ohi],
                                      in_=out_sb[:, olo:ohi])


def _build(n_iters=1, ablate=()):
    fp32 = mybir.dt.float32

    nc = bacc.Bacc("TRN2", target_bir_lowering=False, debug=False,
                   num_devices=N_CORES)

    x_d = nc.declare_dram_parameter("x", [IMG, C, S], fp32, isOutput=False)
    gamma_d = nc.declare_dram_parameter("gamma", [C], fp32, isOutput=False)
    beta_d = nc.declare_dram_parameter("beta", [C], fp32, isOutput=False)
    w_d = nc.declare_dram_parameter("weight", [C, 128 * 9], fp32, isOutput=False)
    y_d = nc.declare_dram_parameter("y", [IMG, C, S], mybir.dt.float16,
                                    isOutput=True)
    params = (x_d, gamma_d, beta_d, w_d, y_d)

    with tile.TileContext(nc) as tc:
        with (
            tc.tile_pool(name="xp", bufs=2) as xp,
            tc.tile_pool(name="apad", bufs=1) as apadp,
            tc.tile_pool(name="wp", bufs=1) as wp,
            tc.tile_pool(name="tmp", bufs=4) as tmpp,
            tc.tile_pool(name="outp", bufs=2) as outp,
            tc.tile_pool(name="small", bufs=1) as smallp,
            tc.tile_pool(name="psum", bufs=1, space="PSUM") as psump,
            tc.tile_pool(name="psmall", bufs=1, space="PSUM") as psmallp,
            tc.tile_pool(name="dram", bufs=4, space="DRAM") as dramp,
        ):
            pools = (xp, apadp, wp, tmpp, outp, smallp, psump, psmallp, dramp)
            pre = _emit_prelude(nc, tc, pools, params)
            st = _emit_front(nc, tc, pools, params, ablate)
            _emit_dispatch(nc, pools, st, ablate)
            for it in range(n_iters):
                nst = (_emit_front(nc, tc, pools, params, ablate)
                       if it + 1 < n_iters else None)
                _emit_back(nc, tc, pools, params, pre, st,
                           pipelined=n_iters > 1, ablate=ablate)
                if nst is not None:
                    _emit_dispatch(nc, pools, nst, ablate)
                st = nst

    nc.finalize()
    return nc


def _get_nc(n_iters=1):
    key = ("nc", n_iters)
    if key not in _CACHE:
        _CACHE[key] = _build(n_iters)
    return _CACHE[key]


def make_in_maps(x, gamma, beta, weight):
    x = np.ascontiguousarray(np.asarray(x, np.float32)).reshape(N_CORES, IMG, C, S)
    w = np.ascontiguousarray(np.asarray(weight, np.float32)).reshape(C, 128 * 9)
    gamma = np.ascontiguousarray(np.asarray(gamma, np.float32))
    beta = np.ascontiguousarray(np.asarray(beta, np.float32))
    return [
        {"x": x[c], "gamma": gamma, "beta": beta, "weight": w}
        for c in range(N_CORES)
    ]


def kernel(x, gamma, beta, weight):
    import os
    from concourse.bass_utils import run_bass_kernel_spmd

    nc = _get_nc()
    in_maps = make_in_maps(x, gamma, beta, weight)
    core_ids = list(range(N_CORES))
    try:
        res = run_bass_kernel_spmd(nc, in_maps, core_ids)
    except ModuleNotFoundError:
        # BASS_TRACE set but no NTFF profile hook in this container
        os.environ["BASS_NEVER_TRACE"] = "1"
        res = run_bass_kernel_spmd(nc, in_maps, core_ids)
    out = np.stack([res.results[c]["y"] for c in range(N_CORES)], axis=0)
    return out.reshape(32, C, HW, HW).astype(np.float32)



# revision 16
# speedup vs baseline: 4.1045x; 1.0755x over previous
"""Trainium2 Bass kernel for BasicQuantConv2d (sync-BN + HWGQ + gauss-quant + 3x3 conv).

Strategy (8 NeuronCores, data-parallel over batch):
  - Each core takes 4 of the 32 images: x shard [4, 128, 56, 56].
  - BN batch stats: per-core bn_stats/bn_aggr -> (mean, E[x^2]) payload,
    AllGather across the 8 cores + local 8-way sum (sync-BN; AllGather is
    ~2x cheaper than AllReduce for a 1KB payload), then the exact /8 is a
    power-of-two scale. Per-channel scale/bias follow.
  - BN + HWGQ folds to ia = RNE_round(clip(x*s_c + b_c, 0, 3)) in {0..3};
    RNE rounding via the fp32 magic constant 1.5*2^23 (matches jnp.round).
  - gauss_quantize(w) == iw * (step/2) with iw in {-3,-1,1,3}; std(w) is
    computed on-device (reduction + ones-matmul broadcast + Newton-refined
    rsqrt), weights transposed per-tap on the PE for the conv lhsT.
  - The 3x3 conv runs in fp8e4m3 (ia in {0..3}, iw in {-3,-1,1,3} are exact
    in fp8; PSUM accumulates fp32 => conv is EXACT integer arithmetic).
    Per output row-chunk: 5 passes -- 3 DoubleRow matmuls (vertical tap
    pairs kh=0&1 per kw, pair-step 64B via the padded row width), 1
    DoubleRow pairing (2,0)+(2,1) horizontally (pair-step 1B), and 1 plain
    fp8 matmul for (2,2) -- accumulated into 7 PSUM banks per image.
  - Output = PSUM * (0.538*step/2) via ScalarE, 896-col DMA out.

Engine assignment (rev 2, microbench-driven): GpSimd/Pool elementwise ops
are EVICTED from the steady state -- concurrent Pool + DVE-2-port traffic
convoys on the shared SBUF port pair (HW-measured 3x blowup) -- so clips
and RNE rounds run on DVE and the BN affine runs on the Activation engine
(Identity activation with per-channel scale/bias APs), leaving Pool only
the collective dispatch.

Pipelining: `_build(n_iters=K)` emits a prelude (weight path, pad memsets,
gamma/beta) once, then K software-pipelined iterations: each loop round
emits the NEXT iteration's loads+stats (front) before this iteration's
reduce/chain/phase C (back), so the in-order DVE queue processes next-
iteration bn_stats during this iteration's collective. x tiles are double-
buffered (bufs=2), all BN affines run before the per-image clip/round/conv
so x buffers release early, DMA queues are split (x loads on sync, outputs
on scalar, payload/gather on gpsimd), and per-image "bridge" PE fillers keep
the tensor engine's p-state ramp hot across inter-image dependency gaps.
Output DMAs move fp16 (host casts back to fp32); that halves the output
stream at <=2^-11 added relative error. test.py measures per-iteration
device time through the ~80ms axon RPC floor.
"""

import os

import numpy as np

import concourse.bacc as bacc
import concourse.bass as bass
import concourse.tile as tile
from concourse import mybir
from concourse.masks import make_identity

N_CORES = 8
IMG = 4            # images per core
C = 128            # channels (= partitions)
HW = 56
S = HW * HW        # 3136 pixels per image
G = 448            # stats/affine granule (8 image rows)
NT = HW // 8       # 7 granules per image
PR = 58            # padded rows
PCW = 64           # padded row width (interior at cols 2..57; pair-step 64B for DoubleRow)
R = 8              # output rows per matmul tile
NFREE = R * HW     # 448 matmul free dim

HWGQ_STEP = 0.538
GAUSS = 0.996
BN_EPS = 1e-3
MAGIC = float(np.float32(1.5 * 2**23))
NW = 128 * 128 * 9          # weight element count

N_FILL = int(os.environ.get("KV1_NF", "14"))    # PE fillers per iteration
N_BRIDGE = int(os.environ.get("KV1_NB", "14"))   # per-image PE bridge fillers
X_NEWTON = 2       # rsqrt Newton iterations for the BN scale (critical path)

NOPOOL = os.environ.get("KV1_NOPOOL", "1") == "1"
AFFACT = os.environ.get("KV1_AFFACT", "1") == "1"
APAR = os.environ.get("KV1_APAR", "1") == "1"   # a_t double-buffer by parity

_CACHE = {}


def _emit_prelude(nc, tc, pools, params):
    """Iteration-invariant work: weight quantization, pad memsets, gamma/beta."""
    fp32 = mybir.dt.float32
    fp8 = mybir.dt.float8e4
    xp, apadp, wp, tmpp, outp, smallp, psump, psmallp, dramp = pools
    x_d, gamma_d, beta_d, w_d, y_d = params
    AF = mybir.ActivationFunctionType
    OP = mybir.AluOpType

    w_sb = wp.tile([C, 128 * 9], fp32)
    nc.sync.dma_start(out=w_sb[:], in_=w_d.ap())

    ident = smallp.tile([C, 128], fp32, tag="ident")
    make_identity(nc, ident[:])

    # one 2KB PSUM bank sliced 4 ways: transpose ping/pong, pg, fillers —
    # disjoint slices so the tile dep tracker doesn't serialize the pipeline
    psm = psmallp.tile([C, 512], fp32, tag="psm", name="psm")

    # transpose each tap: wT[ci, slot, co]; slots pair (kh=0,kw) with (kh=1,kw)
    # adjacently for DoubleRow; kh=2 taps in slots 6..8 ((2,0),(2,1) pair too).
    # slot order: (0,0),(1,0),(0,1),(1,1),(0,2),(1,2),(2,0),(2,1),(2,2)
    SLOT = {(0, 0): 0, (1, 0): 1, (0, 1): 2, (1, 1): 3,
            (0, 2): 4, (1, 2): 5, (2, 0): 6, (2, 1): 7, (2, 2): 8}
    wT = wp.tile([C, 9, 128], fp32)
    w3 = w_sb[:].rearrange("p (ci t) -> p ci t", t=9)
    for t in range(9):
        kh, kw = divmod(t, 3)
        pt = psm[:, (t % 2) * 128:(t % 2) * 128 + 128]
        nc.tensor.transpose(pt, w3[:, :, t], ident[:])
        nc.scalar.copy(out=wT[:, SLOT[(kh, kw)], :], in_=pt)

    # global sum / sumsq of w: ScalarE accum_out row-sums + ones-matmul bcast
    # (scratch shares the uw buffer -- both are prelude-only, used serially)
    uw = wp.tile([C, 9, 128], fp32)
    w2_sb = uw[:].rearrange("p a b -> p (a b)")
    rsums = smallp.tile([C, 2], fp32, tag="rsums")
    nc.scalar.activation(out=w2_sb, in_=w_sb[:], func=AF.Identity,
                         accum_out=rsums[:, 0:1])
    nc.scalar.activation(out=w2_sb, in_=w_sb[:], func=AF.Square,
                         accum_out=rsums[:, 1:2])
    ones = smallp.tile([C, 128], fp32, tag="ones")
    nc.vector.memset(ones[:], 1.0)
    pg = psm[:, 256:384]
    nc.tensor.matmul(pg[:, 0:2], lhsT=ones[:], rhs=rsums[:], start=True, stop=True)
    gs = smallp.tile([C, 2], fp32, tag="gs")
    nc.vector.tensor_copy(gs[:], pg[:, 0:2])

    # wvar = E[w^2] - E[w]^2 ; rw = rsqrt(wvar) Newton-refined
    wmean = smallp.tile([C, 1], fp32, tag="wmean")
    wvar = smallp.tile([C, 1], fp32, tag="wvar")
    nc.vector.tensor_scalar_mul(wmean[:], gs[:, 0:1], 1.0 / NW)
    nc.vector.tensor_scalar_mul(wvar[:], gs[:, 1:2], 1.0 / NW)
    wm2 = smallp.tile([C, 1], fp32, tag="wm2")
    nc.vector.tensor_mul(wm2[:], wmean[:], wmean[:])
    nc.vector.tensor_sub(wvar[:], wvar[:], wm2[:])

    rw = smallp.tile([C, 1], fp32, tag="rw")
    nc.scalar.activation(out=rw[:], in_=wvar[:], func=AF.Sqrt)
    nc.vector.reciprocal(out=rw[:], in_=rw[:])
    tN = smallp.tile([C, 1], fp32, tag="tN")
    for _ in range(2):
        nc.vector.tensor_mul(tN[:], rw[:], rw[:])
        nc.vector.tensor_mul(tN[:], wvar[:], tN[:])
        nc.vector.tensor_scalar(tN[:], tN[:], -0.5, 1.5, OP.mult, OP.add)
        nc.vector.tensor_mul(rw[:], rw[:], tN[:])

    inv_step = smallp.tile([C, 1], fp32, tag="inv_step")
    nc.vector.tensor_scalar_mul(inv_step[:], rw[:], 1.0 / GAUSS)
    # alpha = 0.538 * step/2 = (0.538*0.996/2) * wvar * rw
    alpha = smallp.tile([C, 1], fp32, tag="alpha")
    nc.vector.tensor_mul(alpha[:], wvar[:], rw[:])
    nc.vector.tensor_scalar_mul(alpha[:], alpha[:], HWGQ_STEP * GAUSS / 2.0)

    # quantize transposed weights -> iw in {-3,-1,1,3} (fp8)
    nc.gpsimd.tensor_scalar(uw[:], wT[:], inv_step[:], 0.5, OP.mult, OP.add)
    nc.gpsimd.tensor_scalar(uw[:], uw[:], MAGIC, MAGIC, OP.add, OP.subtract)
    nc.gpsimd.tensor_scalar(uw[:], uw[:], 2.0, -1.0, OP.mult, OP.add)
    wq = wp.tile([C, 9, 128], fp8)
    nc.gpsimd.tensor_scalar(wq[:], uw[:], 3.0, -3.0, OP.min, OP.max)

    # gamma/beta (iteration-invariant)
    gb = smallp.tile([C, 2], fp32, tag="gb")
    gamma_ap = gamma_d.ap().rearrange("(p one) -> p one", one=1)
    beta_ap = beta_d.ap().rearrange("(p one) -> p one", one=1)
    nc.sync.dma_start(out=gb[:, 0:1], in_=gamma_ap)
    nc.sync.dma_start(out=gb[:, 1:2], in_=beta_ap)

    # padded fp8 activation tiles: interior is rewritten every iteration,
    # borders stay zero forever -> memset once here
    npar = 2 if APAR else 1
    a_par = [[apadp.tile([C, PR, PCW], fp8, tag=f"a{p}_{i}", name=f"a_t{p}_{i}")
              for i in range(IMG)] for p in range(npar)]
    for p in range(npar):
        for i in range(IMG):
            nc.gpsimd.memset(a_par[p][i][:], 0.0)

    return dict(a_par=a_par, ones=ones, wq=wq, alpha=alpha, gb=gb, psm=psm)


def _emit_front(nc, tc, pools, params, ablate=()):
    """Loads + BN stats + payload for one iteration (emitted one iteration
    ahead so next-iteration stats fill the DVE queue during this iteration's
    collective)."""
    fp32 = mybir.dt.float32
    xp, apadp, wp, tmpp, outp, smallp, psump, psmallp, dramp = pools
    x_d, gamma_d, beta_d, w_d, y_d = params
    OP = mybir.AluOpType

    # ---------------- load x (896-col tiles, 448-col granule views) --------
    # x loads ride the sync (SP) queue -- a dedicated issuer that is never
    # blocked by compute and holds nothing but loads, so iteration i+1's
    # loads dispatch as soon as their (double-buffered) tiles free up
    xH = [[xp.tile([C, 896 if h < 3 else G], fp32, tag=f"x{i}_{h}",
                   name=f"x{i}_{h}") for h in range(4)] for i in range(IMG)]
    for i in range(IMG):
        for h in range(4):
            lo, hi = h * 896, min((h + 1) * 896, S)
            nc.sync.dma_start(out=xH[i][h][:], in_=x_d.ap()[i][:, lo:hi])

    def xgran(i, g):
        t = xH[i][g // 2]
        if g % 2 == 0:
            return t[:, 0:G]
        return t[:, G:2 * G]

    stats = smallp.tile([C, IMG * NT, 6], fp32)
    for i in range(IMG):
        for g in range(NT):
            nc.vector.bn_stats(out=stats[:, i * NT + g, :], in_=xgran(i, g))
    # payload: (mean, E[x^2]) raw; E[x^2] = var + mean^2 (in-place on pay)
    pay = smallp.tile([C, 2], fp32)
    nc.vector.bn_aggr(out=pay[:], in_=stats[:])
    m2 = smallp.tile([C, 1], fp32)
    nc.vector.tensor_mul(m2[:], pay[:, 0:1], pay[:, 0:1])
    nc.vector.tensor_add(pay[:, 1:2], pay[:, 1:2], m2[:])

    return dict(xH=xH, xgran=xgran, pay=pay)


def _emit_dispatch(nc, pools, st, ablate=()):
    """Collective dispatch (gpsimd queue): payload out, AllGather, gather-in."""
    fp32 = mybir.dt.float32
    xp, apadp, wp, tmpp, outp, smallp, psump, psmallp, dramp = pools
    OP = mybir.AluOpType
    pay = st["pay"]
    # ---------------- sync-BN cross-core exchange ----------------
    cc_in = dramp.tile([C, 2], fp32)
    cc_gath = dramp.tile([N_CORES, C, 2], fp32)
    nc.gpsimd.dma_start(out=cc_in[:], in_=pay[:])
    if "noar" in ablate:
        for r in range(N_CORES):
            nc.gpsimd.dma_start(out=cc_gath[r], in_=cc_in[:])
    else:
        nc.gpsimd.collective_compute(
            "AllGather",
            OP.bypass,
            replica_groups=[list(range(N_CORES))],
            ins=[cc_in.opt()],
            outs=[cc_gath.opt()],
        )
    g_all = smallp.tile([C, N_CORES, 2], fp32)
    nc.gpsimd.dma_start(out=g_all[:], in_=cc_gath[:].rearrange("r p t -> p r t"))
    st["g_all"] = g_all


def _emit_back(nc, tc, pools, params, pre, st, it=0, pipelined=True, ablate=()):
    """Reduce + scale/bias chain + per-image quantize/conv/out."""
    fp32 = mybir.dt.float32
    xp, apadp, wp, tmpp, outp, smallp, psump, psmallp, dramp = pools
    x_d, gamma_d, beta_d, w_d, y_d = params
    AF = mybir.ActivationFunctionType
    OP = mybir.AluOpType
    ones, wq, alpha, gb = (pre["ones"], pre["wq"], pre["alpha"], pre["gb"])
    a_t = pre["a_par"][it % len(pre["a_par"])]
    xH, xgran, g_all = st["xH"], st["xgran"], st["g_all"]

    # local 8-way sum (same order on all cores), then exact /8
    g_sum = smallp.tile([C, 2], fp32)
    nc.vector.tensor_reduce(out=g_sum[:], in_=g_all[:].rearrange("p r t -> p t r"),
                            axis=mybir.AxisListType.X, op=OP.add)

    # PE fillers keep the p-state ramp hot through the collective window;
    # rhs depends on this iteration's first x tile so the scheduler cannot
    # hoist them into the previous iteration's conv burst
    ps_fill = pre["psm"][:, 384:512]
    if "nowarm" not in ablate:
        for _ in range(N_FILL):
            nc.tensor.matmul(ps_fill, lhsT=ones[:], rhs=xH[0][0][:, 0:128],
                             start=True, stop=True)

    # ---------------- global scale/bias ----------------
    # fused but bitwise-identical to the reference chain:
    # me = g_sum*0.125 (exact), vge = (E[x^2] - mean^2) + eps
    me = smallp.tile([C, 2], fp32)      # (global mean, global E[x^2])
    nc.vector.tensor_scalar_mul(me[:], g_sum[:], 0.125)
    meanv = me[:, 0:1]
    vge = smallp.tile([C, 1], fp32)     # var + eps
    gm2 = smallp.tile([C, 1], fp32)
    nc.vector.tensor_mul(gm2[:], meanv, meanv)
    nc.vector.tensor_scalar(vge[:], me[:, 1:2], gm2[:], BN_EPS,
                            OP.subtract, OP.add)
    rx = smallp.tile([C, 1], fp32)
    nc.scalar.activation(out=rx[:], in_=vge[:], func=AF.Sqrt)
    nc.vector.reciprocal(out=rx[:], in_=rx[:])
    tX = smallp.tile([C, 1], fp32)
    for _ in range(X_NEWTON):
        nc.vector.tensor_mul(tX[:], rx[:], rx[:])
        nc.vector.tensor_mul(tX[:], vge[:], tX[:])
        nc.vector.tensor_scalar(tX[:], tX[:], -0.5, 1.5, OP.mult, OP.add)
        nc.vector.tensor_mul(rx[:], rx[:], tX[:])

    # s = gamma * rsqrt / 0.538 ; b = (beta - mean*gamma*rsqrt) / 0.538
    s_q = smallp.tile([C, 1], fp32)
    b_q = smallp.tile([C, 1], fp32)
    ta = smallp.tile([C, 1], fp32)
    nc.vector.tensor_mul(ta[:], gb[:, 0:1], rx[:])          # A = gamma*inv
    nc.vector.tensor_scalar_mul(s_q[:], ta[:], 1.0 / HWGQ_STEP)
    tb = smallp.tile([C, 1], fp32)
    nc.vector.tensor_mul(tb[:], meanv, ta[:])               # mean*A
    nc.vector.tensor_scalar(b_q[:], gb[:, 1:2], tb[:], 1.0 / HWGQ_STEP,
                            OP.subtract, OP.mult)           # (beta-mean*A)/0.538

    # ---------------- per-image quantize + conv ----------------
    # all 28 affines first: x buffers release after ~10us instead of ~17us,
    # which is the binding recurrence for cross-iteration pipelining.
    # Early granules (g<3) on DVE (fast, feeds round chunk 0 quickly),
    # the rest on Pool.
    u_im = []
    for i in range(IMG):
        u_sb = tmpp.tile([C, S], fp32, tag="u", name=f"u_sb{i}")
        u_im.append(u_sb)
        for h in range(4):
            lo, hi = h * 896, min((h + 1) * 896, S)
            if AFFACT:
                nc.scalar.activation(out=u_sb[:, lo:hi], in_=xH[i][h][:],
                                     func=AF.Identity, scale=s_q[:],
                                     bias=b_q[:])
            else:
                nc.vector.tensor_scalar(u_sb[:, lo:hi], xH[i][h][:],
                                        s_q[:], b_q[:], OP.mult, OP.add)
    for i in range(IMG):
        u_sb = u_im[i]
        # clip in place on Pool, then RNE round via MAGIC into the padded
        # fp8 tile (DVE; the last image's rounds go to Pool to shorten the
        # DVE tail, which is the binding engine in steady state)
        for (r0, r1) in ((0, 16), (16, 32), (32, 48), (48, 56)):
            lo, hi = r0 * HW, r1 * HW
            ceng = nc.vector if NOPOOL else nc.gpsimd
            ceng.tensor_scalar(u_sb[:, lo:hi], u_sb[:, lo:hi], 3.0, 0.0,
                               OP.min, OP.max)
            reng = (nc.vector if NOPOOL else
                    (nc.vector if (i < 2 or (i == 2 and r0 < 32))
                     else nc.gpsimd))
            reng.tensor_scalar(a_t[i][:, r0 + 1:r1 + 1, 2:58],
                               u_sb[:, lo:hi].rearrange(
                                   "p (h w) -> p h w", h=r1 - r0),
                               MAGIC, MAGIC, OP.add, OP.subtract)

        # bridge fillers: keep the PE p-state ramp alive across the
        # inter-image a_t dependency gap (dep on this image's clipped u);
        # only useful when iterations pipeline
        for _ in range(N_BRIDGE if pipelined else 0):
            nc.tensor.matmul(pre["psm"][:, 384:512], lhsT=ones[:],
                             rhs=u_sb[:, 0:128], start=True, stop=True)
        out_sb = outp.tile([C, S], mybir.dt.float16, tag="o", name=f"out_sb{i}")
        base = a_t[i][:]
        ps = [psump.tile([C, NFREE], fp32, tag=f"ps{c}", name=f"ps{i}_{c}")
              for c in range(NT)]
        # 5 passes per chunk (cix outer so each PSUM bank completes ASAP):
        # 3 DoubleRow (kh=0&1 per kw), DoubleRow (2,0)+(2,1), single (2,2)
        for cix in range(NT):
            h0 = cix * R
            if "noconv" in ablate:
                continue
            for g in range(5):
                if g < 3:
                    kw = g
                    rhs = bass.AP(
                        tensor=base.tensor,
                        offset=base.offset + h0 * PCW + (kw + 1),
                        ap=[base.ap[0], [PCW, 2], [PCW, R], [1, HW]],
                    )
                    nc.tensor.matmul(ps[cix][:], lhsT=wq[:, 2 * kw: 2 * kw + 2, :],
                                     rhs=rhs, start=(g == 0), stop=False,
                                     perf_mode=mybir.MatmulPerfMode.DoubleRow)
                elif g == 3:
                    rhs = bass.AP(
                        tensor=base.tensor,
                        offset=base.offset + (h0 + 2) * PCW + 1,
                        ap=[base.ap[0], [1, 2], [PCW, R], [1, HW]],
                    )
                    nc.tensor.matmul(ps[cix][:], lhsT=wq[:, 6:8, :],
                                     rhs=rhs, start=False, stop=False,
                                     perf_mode=mybir.MatmulPerfMode.DoubleRow)
                else:
                    rhs = a_t[i][:, h0 + 2: h0 + 2 + R, 3: 3 + HW]
                    nc.tensor.matmul(ps[cix][:], lhsT=wq[:, 8, :], rhs=rhs,
                                     start=False, stop=True)
            # scale out of PSUM on ScalarE (gpsimd cannot read PSUM);
            # fp16 output halves the out-DMA stream (adds <=2^-11 relative
            # rounding, far inside the error budget)
            nc.scalar.activation(out=out_sb[:, h0 * HW: (h0 + R) * HW],
                                 in_=ps[cix][:], func=AF.Identity,
                                 scale=alpha[:])
            # 896-col output DMAs (sync queue) to halve descriptor count
            if cix % 2 == 1 or cix == NT - 1:
                olo = (cix // 2) * 2 * NFREE if cix % 2 == 1 else cix * NFREE
                ohi = (cix + 1) * NFREE
                nc.scalar.dma_start(out=y_d.ap()[i][:, olo:ohi],
                                      in_=out_sb[:, olo:ohi])


def _build(n_iters=1, ablate=()):
    fp32 = mybir.dt.float32

    nc = bacc.Bacc("TRN2", target_bir_lowering=False, debug=False,
                   num_devices=N_CORES)

    x_d = nc.declare_dram_parameter("x", [IMG, C, S], fp32, isOutput=False)
    gamma_d = nc.declare_dram_parameter("gamma", [C], fp32, isOutput=False)
    beta_d = nc.declare_dram_parameter("beta", [C], fp32, isOutput=False)
    w_d = nc.declare_dram_parameter("weight", [C, 128 * 9], fp32, isOutput=False)
    y_d = nc.declare_dram_parameter("y", [IMG, C, S], mybir.dt.float16,
                                    isOutput=True)
    params = (x_d, gamma_d, beta_d, w_d, y_d)

    with tile.TileContext(nc) as tc:
        with (
            tc.tile_pool(name="xp", bufs=2) as xp,
            tc.tile_pool(name="apad", bufs=1) as apadp,
            tc.tile_pool(name="wp", bufs=1) as wp,
            tc.tile_pool(name="tmp", bufs=4) as tmpp,
            tc.tile_pool(name="outp", bufs=2) as outp,
            tc.tile_pool(name="small", bufs=1) as smallp,
            tc.tile_pool(name="psum", bufs=1, space="PSUM") as psump,
            tc.tile_pool(name="psmall", bufs=1, space="PSUM") as psmallp,
            tc.tile_pool(name="dram", bufs=4, space="DRAM") as dramp,
        ):
            pools = (xp, apadp, wp, tmpp, outp, smallp, psump, psmallp, dramp)
            pre = _emit_prelude(nc, tc, pools, params)
            st = _emit_front(nc, tc, pools, params, ablate)
            _emit_dispatch(nc, pools, st, ablate)
            for it in range(n_iters):
                nst = (_emit_front(nc, tc, pools, params, ablate)
                       if it + 1 < n_iters else None)
                _emit_back(nc, tc, pools, params, pre, st, it=it,
                           pipelined=n_iters > 1, ablate=ablate)
                if nst is not None:
                    _emit_dispatch(nc, pools, nst, ablate)
                st = nst

    nc.finalize()
    return nc


def _get_nc(n_iters=1):
    key = ("nc", n_iters)
    if key not in _CACHE:
        _CACHE[key] = _build(n_iters)
    return _CACHE[key]


def make_in_maps(x, gamma, beta, weight):
    x = np.ascontiguousarray(np.asarray(x, np.float32)).reshape(N_CORES, IMG, C, S)
    w = np.ascontiguousarray(np.asarray(weight, np.float32)).reshape(C, 128 * 9)
    gamma = np.ascontiguousarray(np.asarray(gamma, np.float32))
    beta = np.ascontiguousarray(np.asarray(beta, np.float32))
    return [
        {"x": x[c], "gamma": gamma, "beta": beta, "weight": w}
        for c in range(N_CORES)
    ]


def kernel(x, gamma, beta, weight):
    import os
    from concourse.bass_utils import run_bass_kernel_spmd

    nc = _get_nc()
    in_maps = make_in_maps(x, gamma, beta, weight)
    core_ids = list(range(N_CORES))
    try:
        res = run_bass_kernel_spmd(nc, in_maps, core_ids)
    except ModuleNotFoundError:
        # BASS_TRACE set but no NTFF profile hook in this container
        os.environ["BASS_NEVER_TRACE"] = "1"
        res = run_bass_kernel_spmd(nc, in_maps, core_ids)
    out = np.stack([res.results[c]["y"] for c in range(N_CORES)], axis=0)
    return out.reshape(32, C, HW, HW).astype(np.float32)



# revision 17
# speedup vs baseline: 5.1480x; 1.2542x over previous
"""Trainium2 Bass kernel for BasicQuantConv2d (sync-BN + HWGQ + gauss-quant + 3x3 conv).

Strategy (8 NeuronCores, data-parallel over batch):
  - Each core takes 4 of the 32 images: x shard [4, 128, 56, 56].
  - BN batch stats: per-core bn_stats/bn_aggr -> (mean, E[x^2]) payload,
    AllGather across the 8 cores + local 8-way sum (sync-BN; AllGather is
    ~2x cheaper than AllReduce for a 1KB payload), then the exact /8 is a
    power-of-two scale. Per-channel scale/bias follow.
  - BN + HWGQ folds to ia = RNE_round(clip(x*s_c + b_c, 0, 3)) in {0..3};
    RNE rounding via the fp32 magic constant 1.5*2^23 (matches jnp.round).
  - gauss_quantize(w) == iw * (step/2) with iw in {-3,-1,1,3}; std(w) is
    computed on-device (reduction + ones-matmul broadcast + Newton-refined
    rsqrt), weights transposed per-tap on the PE for the conv lhsT.
  - The 3x3 conv runs in fp8e4m3 (ia in {0..3}, iw in {-3,-1,1,3} are exact
    in fp8; PSUM accumulates fp32 => conv is EXACT integer arithmetic).
    Per output row-chunk: 5 passes -- 3 DoubleRow matmuls (vertical tap
    pairs kh=0&1 per kw, pair-step 64B via the padded row width), 1
    DoubleRow pairing (2,0)+(2,1) horizontally (pair-step 1B), and 1 plain
    fp8 matmul for (2,2) -- accumulated into 7 PSUM banks per image.
  - Output = PSUM * (0.538*step/2) via ScalarE, 896-col DMA out.

Engine assignment (rev 2, microbench-driven): GpSimd/Pool elementwise ops
are EVICTED from the steady state -- concurrent Pool + DVE-2-port traffic
convoys on the shared SBUF port pair (HW-measured 3x blowup) -- so clips
and RNE rounds run on DVE and the BN affine runs on the Activation engine
(Identity activation with per-channel scale/bias APs), leaving Pool only
the collective dispatch.

Pipelining: `_build(n_iters=K)` emits a prelude (weight path, pad memsets,
gamma/beta) once, then K software-pipelined iterations: each loop round
emits the NEXT iteration's loads+stats (front) before this iteration's
reduce/chain/phase C (back), so the in-order DVE queue processes next-
iteration bn_stats during this iteration's collective. x tiles are double-
buffered (bufs=2), all BN affines run before the per-image clip/round/conv
so x buffers release early, DMA queues are split (x loads on sync, outputs
on scalar, payload/gather on gpsimd), and per-image "bridge" PE fillers keep
the tensor engine's p-state ramp hot across inter-image dependency gaps.
Output DMAs move fp16 (host casts back to fp32); that halves the output
stream at <=2^-11 added relative error. test.py measures per-iteration
device time through the ~80ms axon RPC floor.
"""

import os

import numpy as np

import concourse.bacc as bacc
import concourse.bass as bass
import concourse.tile as tile
from concourse import mybir
from concourse.masks import make_identity

N_CORES = 8
IMG = 4            # images per core
C = 128            # channels (= partitions)
HW = 56
S = HW * HW        # 3136 pixels per image
G = 448            # stats/affine granule (8 image rows)
NT = HW // 8       # 7 granules per image
PR = 58            # padded rows
PCW = 64           # padded row width (interior at cols 2..57; pair-step 64B for DoubleRow)
R = 8              # output rows per matmul tile
NFREE = R * HW     # 448 matmul free dim

HWGQ_STEP = 0.538
GAUSS = 0.996
BN_EPS = 1e-3
MAGIC = float(np.float32(1.5 * 2**23))
NW = 128 * 128 * 9          # weight element count

N_FILL = int(os.environ.get("KV1_NF", "14"))    # PE fillers per iteration
N_BRIDGE = int(os.environ.get("KV1_NB", "14"))   # per-image PE bridge fillers
X_NEWTON = 2       # rsqrt Newton iterations for the BN scale (critical path)

NOPOOL = os.environ.get("KV1_NOPOOL", "1") == "1"
AFFACT = os.environ.get("KV1_AFFACT", "1") == "1"
APAR = os.environ.get("KV1_APAR", "1") == "1"   # a_t double-buffer by parity
MERGE = os.environ.get("KV1_MERGE", "0") == "1"  # whole-image clip/round instrs

_CACHE = {}


def _emit_prelude(nc, tc, pools, params):
    """Iteration-invariant work: weight quantization, pad memsets, gamma/beta."""
    fp32 = mybir.dt.float32
    fp8 = mybir.dt.float8e4
    xp, apadp, wp, tmpp, outp, smallp, psump, psmallp, dramp = pools
    x_d, gamma_d, beta_d, w_d, y_d = params
    AF = mybir.ActivationFunctionType
    OP = mybir.AluOpType

    w_sb = wp.tile([C, 128 * 9], fp32)
    nc.sync.dma_start(out=w_sb[:], in_=w_d.ap())

    ident = smallp.tile([C, 128], fp32, tag="ident")
    make_identity(nc, ident[:])

    # one 2KB PSUM bank sliced 4 ways: transpose ping/pong, pg, fillers —
    # disjoint slices so the tile dep tracker doesn't serialize the pipeline
    psm = psmallp.tile([C, 512], fp32, tag="psm", name="psm")

    # transpose each tap: wT[ci, slot, co]; slots pair (kh=0,kw) with (kh=1,kw)
    # adjacently for DoubleRow; kh=2 taps in slots 6..8 ((2,0),(2,1) pair too).
    # slot order: (0,0),(1,0),(0,1),(1,1),(0,2),(1,2),(2,0),(2,1),(2,2)
    SLOT = {(0, 0): 0, (1, 0): 1, (0, 1): 2, (1, 1): 3,
            (0, 2): 4, (1, 2): 5, (2, 0): 6, (2, 1): 7, (2, 2): 8}
    wT = wp.tile([C, 9, 128], fp32)
    w3 = w_sb[:].rearrange("p (ci t) -> p ci t", t=9)
    for t in range(9):
        kh, kw = divmod(t, 3)
        pt = psm[:, (t % 2) * 128:(t % 2) * 128 + 128]
        nc.tensor.transpose(pt, w3[:, :, t], ident[:])
        nc.scalar.copy(out=wT[:, SLOT[(kh, kw)], :], in_=pt)

    # global sum / sumsq of w: ScalarE accum_out row-sums + ones-matmul bcast
    # (scratch shares the uw buffer -- both are prelude-only, used serially)
    uw = wp.tile([C, 9, 128], fp32)
    w2_sb = uw[:].rearrange("p a b -> p (a b)")
    rsums = smallp.tile([C, 2], fp32, tag="rsums")
    nc.scalar.activation(out=w2_sb, in_=w_sb[:], func=AF.Identity,
                         accum_out=rsums[:, 0:1])
    nc.scalar.activation(out=w2_sb, in_=w_sb[:], func=AF.Square,
                         accum_out=rsums[:, 1:2])
    ones = smallp.tile([C, 128], fp32, tag="ones")
    nc.vector.memset(ones[:], 1.0)
    pg = psm[:, 256:384]
    nc.tensor.matmul(pg[:, 0:2], lhsT=ones[:], rhs=rsums[:], start=True, stop=True)
    gs = smallp.tile([C, 2], fp32, tag="gs")
    nc.vector.tensor_copy(gs[:], pg[:, 0:2])

    # wvar = E[w^2] - E[w]^2 ; rw = rsqrt(wvar) Newton-refined
    wmean = smallp.tile([C, 1], fp32, tag="wmean")
    wvar = smallp.tile([C, 1], fp32, tag="wvar")
    nc.vector.tensor_scalar_mul(wmean[:], gs[:, 0:1], 1.0 / NW)
    nc.vector.tensor_scalar_mul(wvar[:], gs[:, 1:2], 1.0 / NW)
    wm2 = smallp.tile([C, 1], fp32, tag="wm2")
    nc.vector.tensor_mul(wm2[:], wmean[:], wmean[:])
    nc.vector.tensor_sub(wvar[:], wvar[:], wm2[:])

    rw = smallp.tile([C, 1], fp32, tag="rw")
    nc.scalar.activation(out=rw[:], in_=wvar[:], func=AF.Sqrt)
    nc.vector.reciprocal(out=rw[:], in_=rw[:])
    tN = smallp.tile([C, 1], fp32, tag="tN")
    for _ in range(2):
        nc.vector.tensor_mul(tN[:], rw[:], rw[:])
        nc.vector.tensor_mul(tN[:], wvar[:], tN[:])
        nc.vector.tensor_scalar(tN[:], tN[:], -0.5, 1.5, OP.mult, OP.add)
        nc.vector.tensor_mul(rw[:], rw[:], tN[:])

    inv_step = smallp.tile([C, 1], fp32, tag="inv_step")
    nc.vector.tensor_scalar_mul(inv_step[:], rw[:], 1.0 / GAUSS)
    # alpha = 0.538 * step/2 = (0.538*0.996/2) * wvar * rw
    alpha = smallp.tile([C, 1], fp32, tag="alpha")
    nc.vector.tensor_mul(alpha[:], wvar[:], rw[:])
    nc.vector.tensor_scalar_mul(alpha[:], alpha[:], HWGQ_STEP * GAUSS / 2.0)

    # quantize transposed weights -> iw in {-3,-1,1,3} (fp8)
    nc.gpsimd.tensor_scalar(uw[:], wT[:], inv_step[:], 0.5, OP.mult, OP.add)
    nc.gpsimd.tensor_scalar(uw[:], uw[:], MAGIC, MAGIC, OP.add, OP.subtract)
    nc.gpsimd.tensor_scalar(uw[:], uw[:], 2.0, -1.0, OP.mult, OP.add)
    wq = wp.tile([C, 9, 128], fp8)
    nc.gpsimd.tensor_scalar(wq[:], uw[:], 3.0, -3.0, OP.min, OP.max)

    # gamma/beta (iteration-invariant)
    gb = smallp.tile([C, 2], fp32, tag="gb")
    gamma_ap = gamma_d.ap().rearrange("(p one) -> p one", one=1)
    beta_ap = beta_d.ap().rearrange("(p one) -> p one", one=1)
    nc.sync.dma_start(out=gb[:, 0:1], in_=gamma_ap)
    nc.sync.dma_start(out=gb[:, 1:2], in_=beta_ap)

    # padded fp8 activation tiles: interior is rewritten every iteration,
    # borders stay zero forever -> memset once here
    npar = 2 if APAR else 1
    a_par = [[apadp.tile([C, PR, PCW], fp8, tag=f"a{p}_{i}", name=f"a_t{p}_{i}")
              for i in range(IMG)] for p in range(npar)]
    for p in range(npar):
        for i in range(IMG):
            nc.gpsimd.memset(a_par[p][i][:], 0.0)

    return dict(a_par=a_par, ones=ones, wq=wq, alpha=alpha, gb=gb, psm=psm)


def _emit_front(nc, tc, pools, params, ablate=()):
    """Loads + BN stats + payload for one iteration (emitted one iteration
    ahead so next-iteration stats fill the DVE queue during this iteration's
    collective)."""
    fp32 = mybir.dt.float32
    xp, apadp, wp, tmpp, outp, smallp, psump, psmallp, dramp = pools
    x_d, gamma_d, beta_d, w_d, y_d = params
    OP = mybir.AluOpType

    # ---------------- load x (896-col tiles, 448-col granule views) --------
    # x loads ride the sync (SP) queue -- a dedicated issuer that is never
    # blocked by compute and holds nothing but loads, so iteration i+1's
    # loads dispatch as soon as their (double-buffered) tiles free up
    xH = [[xp.tile([C, 896 if h < 3 else G], fp32, tag=f"x{i}_{h}",
                   name=f"x{i}_{h}") for h in range(4)] for i in range(IMG)]
    for i in range(IMG):
        for h in range(4):
            lo, hi = h * 896, min((h + 1) * 896, S)
            nc.sync.dma_start(out=xH[i][h][:], in_=x_d.ap()[i][:, lo:hi])

    def xgran(i, g):
        t = xH[i][g // 2]
        if g % 2 == 0:
            return t[:, 0:G]
        return t[:, G:2 * G]

    stats = smallp.tile([C, IMG * NT, 6], fp32)
    for i in range(IMG):
        for g in range(NT):
            nc.vector.bn_stats(out=stats[:, i * NT + g, :], in_=xgran(i, g))
    # payload: (mean, E[x^2]) raw; E[x^2] = var + mean^2 (in-place on pay)
    pay = smallp.tile([C, 2], fp32)
    nc.vector.bn_aggr(out=pay[:], in_=stats[:])
    m2 = smallp.tile([C, 1], fp32)
    nc.vector.tensor_mul(m2[:], pay[:, 0:1], pay[:, 0:1])
    nc.vector.tensor_add(pay[:, 1:2], pay[:, 1:2], m2[:])

    return dict(xH=xH, xgran=xgran, pay=pay)


def _emit_dispatch(nc, pools, st, ablate=()):
    """Collective dispatch (gpsimd queue): payload out, AllGather, gather-in."""
    fp32 = mybir.dt.float32
    xp, apadp, wp, tmpp, outp, smallp, psump, psmallp, dramp = pools
    OP = mybir.AluOpType
    pay = st["pay"]
    # ---------------- sync-BN cross-core exchange ----------------
    cc_in = dramp.tile([C, 2], fp32)
    cc_gath = dramp.tile([N_CORES, C, 2], fp32)
    nc.gpsimd.dma_start(out=cc_in[:], in_=pay[:])
    if "noar" in ablate:
        for r in range(N_CORES):
            nc.gpsimd.dma_start(out=cc_gath[r], in_=cc_in[:])
    else:
        nc.gpsimd.collective_compute(
            "AllGather",
            OP.bypass,
            replica_groups=[list(range(N_CORES))],
            ins=[cc_in.opt()],
            outs=[cc_gath.opt()],
        )
    g_all = smallp.tile([C, N_CORES, 2], fp32)
    nc.gpsimd.dma_start(out=g_all[:], in_=cc_gath[:].rearrange("r p t -> p r t"))
    st["g_all"] = g_all


def _emit_back(nc, tc, pools, params, pre, st, it=0, pipelined=True, ablate=()):
    """Reduce + scale/bias chain + per-image quantize/conv/out."""
    fp32 = mybir.dt.float32
    xp, apadp, wp, tmpp, outp, smallp, psump, psmallp, dramp = pools
    x_d, gamma_d, beta_d, w_d, y_d = params
    AF = mybir.ActivationFunctionType
    OP = mybir.AluOpType
    ones, wq, alpha, gb = (pre["ones"], pre["wq"], pre["alpha"], pre["gb"])
    a_t = pre["a_par"][it % len(pre["a_par"])]
    xH, xgran, g_all = st["xH"], st["xgran"], st["g_all"]

    # local 8-way sum (same order on all cores), then exact /8
    g_sum = smallp.tile([C, 2], fp32)
    nc.vector.tensor_reduce(out=g_sum[:], in_=g_all[:].rearrange("p r t -> p t r"),
                            axis=mybir.AxisListType.X, op=OP.add)

    # PE fillers keep the p-state ramp hot through the collective window;
    # rhs depends on this iteration's first x tile so the scheduler cannot
    # hoist them into the previous iteration's conv burst
    ps_fill = pre["psm"][:, 384:512]
    if "nowarm" not in ablate:
        for _ in range(N_FILL):
            nc.tensor.matmul(ps_fill, lhsT=ones[:], rhs=xH[0][0][:, 0:128],
                             start=True, stop=True)

    # ---------------- global scale/bias ----------------
    # fused but bitwise-identical to the reference chain:
    # me = g_sum*0.125 (exact), vge = (E[x^2] - mean^2) + eps
    me = smallp.tile([C, 2], fp32)      # (global mean, global E[x^2])
    nc.vector.tensor_scalar_mul(me[:], g_sum[:], 0.125)
    meanv = me[:, 0:1]
    vge = smallp.tile([C, 1], fp32)     # var + eps
    gm2 = smallp.tile([C, 1], fp32)
    nc.vector.tensor_mul(gm2[:], meanv, meanv)
    nc.vector.tensor_scalar(vge[:], me[:, 1:2], gm2[:], BN_EPS,
                            OP.subtract, OP.add)
    rx = smallp.tile([C, 1], fp32)
    nc.scalar.activation(out=rx[:], in_=vge[:], func=AF.Sqrt)
    nc.vector.reciprocal(out=rx[:], in_=rx[:])
    tX = smallp.tile([C, 1], fp32)
    for _ in range(X_NEWTON):
        nc.vector.tensor_mul(tX[:], rx[:], rx[:])
        nc.vector.tensor_mul(tX[:], vge[:], tX[:])
        nc.vector.tensor_scalar(tX[:], tX[:], -0.5, 1.5, OP.mult, OP.add)
        nc.vector.tensor_mul(rx[:], rx[:], tX[:])

    # s = gamma * rsqrt / 0.538 ; b = (beta - mean*gamma*rsqrt) / 0.538
    s_q = smallp.tile([C, 1], fp32)
    b_q = smallp.tile([C, 1], fp32)
    ta = smallp.tile([C, 1], fp32)
    nc.vector.tensor_mul(ta[:], gb[:, 0:1], rx[:])          # A = gamma*inv
    nc.vector.tensor_scalar_mul(s_q[:], ta[:], 1.0 / HWGQ_STEP)
    tb = smallp.tile([C, 1], fp32)
    nc.vector.tensor_mul(tb[:], meanv, ta[:])               # mean*A
    nc.vector.tensor_scalar(b_q[:], gb[:, 1:2], tb[:], 1.0 / HWGQ_STEP,
                            OP.subtract, OP.mult)           # (beta-mean*A)/0.538

    # ---------------- per-image quantize + conv ----------------
    # all 28 affines first: x buffers release after ~10us instead of ~17us,
    # which is the binding recurrence for cross-iteration pipelining.
    # Early granules (g<3) on DVE (fast, feeds round chunk 0 quickly),
    # the rest on Pool.
    u_im = []
    for i in range(IMG):
        u_sb = tmpp.tile([C, S], fp32, tag="u", name=f"u_sb{i}")
        u_im.append(u_sb)
        for h in range(4):
            lo, hi = h * 896, min((h + 1) * 896, S)
            if AFFACT:
                nc.scalar.activation(out=u_sb[:, lo:hi], in_=xH[i][h][:],
                                     func=AF.Identity, scale=s_q[:],
                                     bias=b_q[:])
            else:
                nc.vector.tensor_scalar(u_sb[:, lo:hi], xH[i][h][:],
                                        s_q[:], b_q[:], OP.mult, OP.add)
    for i in range(IMG):
        u_sb = u_im[i]
        # clip in place on Pool, then RNE round via MAGIC into the padded
        # fp8 tile (DVE; the last image's rounds go to Pool to shorten the
        # DVE tail, which is the binding engine in steady state)
        blocks = (((0, 56),) if MERGE
                  else ((0, 16), (16, 32), (32, 48), (48, 56)))
        for (r0, r1) in blocks:
            lo, hi = r0 * HW, r1 * HW
            ceng = nc.vector if NOPOOL else nc.gpsimd
            ceng.tensor_scalar(u_sb[:, lo:hi], u_sb[:, lo:hi], 3.0, 0.0,
                               OP.min, OP.max)
            reng = (nc.vector if NOPOOL else
                    (nc.vector if (i < 2 or (i == 2 and r0 < 32))
                     else nc.gpsimd))
            reng.tensor_scalar(a_t[i][:, r0 + 1:r1 + 1, 2:58],
                               u_sb[:, lo:hi].rearrange(
                                   "p (h w) -> p h w", h=r1 - r0),
                               MAGIC, MAGIC, OP.add, OP.subtract)

        # bridge fillers: keep the PE p-state ramp alive across the
        # inter-image a_t dependency gap (dep on this image's clipped u);
        # only useful when iterations pipeline
        for _ in range(N_BRIDGE if pipelined else 0):
            nc.tensor.matmul(pre["psm"][:, 384:512], lhsT=ones[:],
                             rhs=u_sb[:, 0:128], start=True, stop=True)
        out_sb = outp.tile([C, S], mybir.dt.float16, tag="o", name=f"out_sb{i}")
        base = a_t[i][:]
        ps = [psump.tile([C, NFREE], fp32, tag=f"ps{c}", name=f"ps{i}_{c}")
              for c in range(NT)]
        # 5 passes per chunk (cix outer so each PSUM bank completes ASAP):
        # 3 DoubleRow (kh=0&1 per kw), DoubleRow (2,0)+(2,1), single (2,2)
        for cix in range(NT):
            h0 = cix * R
            if "noconv" in ablate:
                continue
            for g in range(5):
                if g < 3:
                    kw = g
                    rhs = bass.AP(
                        tensor=base.tensor,
                        offset=base.offset + h0 * PCW + (kw + 1),
                        ap=[base.ap[0], [PCW, 2], [PCW, R], [1, HW]],
                    )
                    nc.tensor.matmul(ps[cix][:], lhsT=wq[:, 2 * kw: 2 * kw + 2, :],
                                     rhs=rhs, start=(g == 0), stop=False,
                                     perf_mode=mybir.MatmulPerfMode.DoubleRow)
                elif g == 3:
                    rhs = bass.AP(
                        tensor=base.tensor,
                        offset=base.offset + (h0 + 2) * PCW + 1,
                        ap=[base.ap[0], [1, 2], [PCW, R], [1, HW]],
                    )
                    nc.tensor.matmul(ps[cix][:], lhsT=wq[:, 6:8, :],
                                     rhs=rhs, start=False, stop=False,
                                     perf_mode=mybir.MatmulPerfMode.DoubleRow)
                else:
                    rhs = a_t[i][:, h0 + 2: h0 + 2 + R, 3: 3 + HW]
                    nc.tensor.matmul(ps[cix][:], lhsT=wq[:, 8, :], rhs=rhs,
                                     start=False, stop=True)
            # scale out of PSUM on ScalarE (gpsimd cannot read PSUM);
            # fp16 output halves the out-DMA stream (adds <=2^-11 relative
            # rounding, far inside the error budget)
            nc.scalar.activation(out=out_sb[:, h0 * HW: (h0 + R) * HW],
                                 in_=ps[cix][:], func=AF.Identity,
                                 scale=alpha[:])
            # 896-col output DMAs (sync queue) to halve descriptor count
            if cix % 2 == 1 or cix == NT - 1:
                olo = (cix // 2) * 2 * NFREE if cix % 2 == 1 else cix * NFREE
                ohi = (cix + 1) * NFREE
                nc.scalar.dma_start(out=y_d.ap()[i][:, olo:ohi],
                                      in_=out_sb[:, olo:ohi])


def _build(n_iters=1, ablate=()):
    fp32 = mybir.dt.float32

    nc = bacc.Bacc("TRN2", target_bir_lowering=False, debug=False,
                   num_devices=N_CORES)

    x_d = nc.declare_dram_parameter("x", [IMG, C, S], fp32, isOutput=False)
    gamma_d = nc.declare_dram_parameter("gamma", [C], fp32, isOutput=False)
    beta_d = nc.declare_dram_parameter("beta", [C], fp32, isOutput=False)
    w_d = nc.declare_dram_parameter("weight", [C, 128 * 9], fp32, isOutput=False)
    y_d = nc.declare_dram_parameter("y", [IMG, C, S], mybir.dt.float16,
                                    isOutput=True)
    params = (x_d, gamma_d, beta_d, w_d, y_d)

    with tile.TileContext(nc) as tc:
        with (
            tc.tile_pool(name="xp", bufs=2) as xp,
            tc.tile_pool(name="apad", bufs=1) as apadp,
            tc.tile_pool(name="wp", bufs=1) as wp,
            tc.tile_pool(name="tmp", bufs=4) as tmpp,
            tc.tile_pool(name="outp", bufs=2) as outp,
            tc.tile_pool(name="small", bufs=1) as smallp,
            tc.tile_pool(name="psum", bufs=1, space="PSUM") as psump,
            tc.tile_pool(name="psmall", bufs=1, space="PSUM") as psmallp,
            tc.tile_pool(name="dram", bufs=4, space="DRAM") as dramp,
        ):
            pools = (xp, apadp, wp, tmpp, outp, smallp, psump, psmallp, dramp)
            pre = _emit_prelude(nc, tc, pools, params)
            st = _emit_front(nc, tc, pools, params, ablate)
            _emit_dispatch(nc, pools, st, ablate)
            for it in range(n_iters):
                nst = (_emit_front(nc, tc, pools, params, ablate)
                       if it + 1 < n_iters else None)
                _emit_back(nc, tc, pools, params, pre, st, it=it,
                           pipelined=n_iters > 1, ablate=ablate)
                if nst is not None:
                    _emit_dispatch(nc, pools, nst, ablate)
                st = nst

    nc.finalize()
    return nc


def _get_nc(n_iters=1):
    key = ("nc", n_iters)
    if key not in _CACHE:
        _CACHE[key] = _build(n_iters)
    return _CACHE[key]


def make_in_maps(x, gamma, beta, weight):
    x = np.ascontiguousarray(np.asarray(x, np.float32)).reshape(N_CORES, IMG, C, S)
    w = np.ascontiguousarray(np.asarray(weight, np.float32)).reshape(C, 128 * 9)
    gamma = np.ascontiguousarray(np.asarray(gamma, np.float32))
    beta = np.ascontiguousarray(np.asarray(beta, np.float32))
    return [
        {"x": x[c], "gamma": gamma, "beta": beta, "weight": w}
        for c in range(N_CORES)
    ]


def kernel(x, gamma, beta, weight):
    import os
    from concourse.bass_utils import run_bass_kernel_spmd

    nc = _get_nc()
    in_maps = make_in_maps(x, gamma, beta, weight)
    core_ids = list(range(N_CORES))
    try:
        res = run_bass_kernel_spmd(nc, in_maps, core_ids)
    except ModuleNotFoundError:
        # BASS_TRACE set but no NTFF profile hook in this container
        os.environ["BASS_NEVER_TRACE"] = "1"
        res = run_bass_kernel_spmd(nc, in_maps, core_ids)
    out = np.stack([res.results[c]["y"] for c in range(N_CORES)], axis=0)
    return out.reshape(32, C, HW, HW).astype(np.float32)

